# revision 7
# baseline (speedup 1.0000x reference)
"""Multi-head attention (B=1, S=4096, D=512, H=8, HD=64) on 8 trn2 NeuronCores.

Sharding: one head per core. Each core projects Q/K/V for its head from a
bf16 copy of x^T, runs attention, applies its head's output projection, and
writes a full [S, D] fp32 partial; the host sums the 8 partials.

Structure (per core):
- All matmuls bf16 (fp32 PSUM accumulation). wq is pre-scaled by 1/sqrt(HD)
  on the host and packed with wk into one [D, 128] tensor so Q^T and K^T come
  out of one matmul stream ([128, 512] PSUM tiles: rows 0-63 Q^T, 64-127 K^T).
- Scores^T tiles [128 t, 512 s]: lhsT = K^T tile (stationary), rhs = Q^T.
- exp splits between ACT (exact, bf16 out) and DVE (Schraudolph: one
  tensor_scalar producing int16 bits that are the bf16 encoding of e^s).
- O accumulation is E-stationary: o_ps[s-tile, 65] += E_tile^T @ [V | 1];
  the softmax denominator Z lands on column 64, per-s on partitions.
- Epilogue per s-chunk: recip(Z), O -> bf16 (padded to 128 cols), DMA
  transpose to O^T, y_ps = O^T.T @ wp, scale by 1/Z into SBUF, DMA out.
  All deferred into the next chunk's instruction stream.
- Chunk 0 of the attention loop is interleaved with the projection phase
  (each score tile only needs K^T tiles already projected), hiding the
  x-load and projection behind chunk-0 lane work.
"""

import numpy as np
import ml_dtypes

import concourse.bacc as bacc
import concourse.mybir as mybir
import concourse.tile as tile
from concourse.bass_utils import run_bass_kernel_spmd

S = 4096
D = 512
HD = 64
H = 8
P = 128
KT = D // P            # 4 c-tiles
NB = S // 512          # 8 s-blocks / s-chunks
NT = S // P            # 32 t-tiles
NST = 512 // P         # 4 s-tiles per chunk
SCALE = HD ** -0.5

F32 = mybir.dt.float32
BF16 = mybir.dt.bfloat16
I16 = mybir.dt.int16

EXP_L = 128.0 / float(np.log(2.0))     # schraudolph multiplier for bf16 bits
EXP_C = 16256.0 - 5.5                  # bias (127<<7), centered


def build_kernel(n_dve=15, ebufs=37, n_warm=6, popslot=4, bulkslot=2, loff=0, dpops=0):
    """n_dve: DVE (approx-exp) t-tiles out of NT=32 per chunk."""
    nc = bacc.Bacc(
        "TRN2",
        target_bir_lowering=False,
        debug=False,
        enable_asserts=False,
        num_devices=H,
    )

    xt = nc.dram_tensor("xt", [D, S], BF16, kind="ExternalInput").ap()
    wqk = nc.dram_tensor("wqk", [D, P], BF16, kind="ExternalInput").ap()
    wv = nc.dram_tensor("wv", [D, HD], BF16, kind="ExternalInput").ap()
    wp = nc.dram_tensor("wp", [HD, D], BF16, kind="ExternalInput").ap()
    ident = nc.dram_tensor("ident", [P, 4 * P], BF16, kind="ExternalInput").ap()
    y = nc.dram_tensor("y", [S, D], BF16, kind="ExternalOutput").ap()

    Exp = mybir.ActivationFunctionType.Exp
    Mult = mybir.AluOpType.mult
    Add = mybir.AluOpType.add

    # lane pattern per chunk: n_dve DVE t-tiles spread among ACT t-tiles
    lanes = ["ACT"] * NT
    if n_dve > 0:
        step = NT / n_dve
        for i in range(n_dve):
            lanes[min(NT - 1, int(i * step) + loff)] = "DVE"

    with tile.TileContext(nc) as tc:
        with (
            tc.tile_pool(name="const", bufs=1) as cp,
            tc.tile_pool(name="ea", bufs=ebufs) as eap,
            tc.tile_pool(name="ed", bufs=ebufs) as edp,
            tc.tile_pool(name="ob", bufs=2) as obp,
            tc.tile_pool(name="otb", bufs=2) as otbp,
            tc.tile_pool(name="ys", bufs=3) as ysp,
            tc.tile_pool(name="xtp", bufs=1) as xtp,
            tc.tile_pool(name="sp", bufs=4, space="PSUM") as sp,
            tc.tile_pool(name="op", bufs=2, space="PSUM") as op,
            tc.tile_pool(name="yp", bufs=1, space="PSUM") as yp,
            tc.tile_pool(name="trp", bufs=1, space="PSUM") as trp,
        ):
            wqk_sb = cp.tile([P, KT, P], BF16, tag="wqk")
            id_sb = cp.tile([P, 4 * P], BF16, tag="ident")
            wv_sb = cp.tile([P, KT, HD], BF16, tag="wv")
            wp_sb = cp.tile([HD, D], BF16, tag="wp")
            qq = cp.tile([HD, S], BF16, tag="qq")      # Q^T (pre-scaled)
            kk = cp.tile([HD, S], BF16, tag="kk")      # K^T
            v_sb = cp.tile([P, NT, HD + 1], BF16, tag="v")
            rz_sb = cp.tile([P, NT, 1], F32, tag="rz")  # 1/Z per s-tile
            xt_sb = xtp.tile([P, KT, S], BF16, tag="xt")

            wsrc = cp.tile([P, 512], BF16, tag="wsrc")
            nc.gpsimd.memset(wsrc, 1.0)
            xt_r = xt.rearrange("(a p) s -> p a s", p=P)
            nc.sync.dma_start(xt_sb[:, :, 0:512], xt_r[:, :, 0:512])
            nc.sync.dma_start(wqk_sb, wqk.rearrange("(a p) d -> p a d", p=P))
            nc.sync.dma_start(id_sb, ident)
            nc.sync.dma_start(wv_sb, wv.rearrange("(a p) d -> p a d", p=P))
            for b in range(1, NB):
                ssl = slice(b * 512, (b + 1) * 512)
                nc.sync.dma_start(xt_sb[:, :, ssl], xt_r[:, :, ssl])
            nc.sync.dma_start(wp_sb, wp)
            nc.gpsimd.memset(v_sb[:, :, HD : HD + 1], 1.0)

            # PE warm-up: matmuls on a memset tile need no DMA, so the PE
            # clock ramp starts immediately while the first x block lands.
            for _ in range(n_warm):
                warm_ps = sp.tile([P, 512], F32, tag="s_ps", name="warm_ps")
                nc.tensor.matmul(
                    warm_ps, wsrc[:, :P], wsrc, start=True, stop=True
                )

            deferred = []
            epi = {}

            def epi_bulk(c, o_ps):
                o_sb = obp.tile([P, NST, P], BF16, tag="ob", name="o_sb")
                ot_sb = otbp.tile([P, NST, P], BF16, tag="otb", name="ot_sb")
                nc.vector.reciprocal(
                    rz_sb[:, c * NST : (c + 1) * NST, 0], o_ps[:, :, HD]
                )
                nc.vector.tensor_copy(o_sb[:, :, :HD], o_ps[:, :, :HD])
                for st in range(NST):
                    nc.sync.dma_start(
                        ot_sb[:, st, :], o_sb[:, st, :], transpose=True
                    )

                def make_part2(st):
                    def part2():
                        y_ps = yp.tile([P, D], F32, tag="y_ps")
                        nc.tensor.matmul(
                            y_ps, ot_sb[:HD, st, :], wp_sb,
                            start=True, stop=True,
                        )
                        ys = ysp.tile([P, D], BF16, tag="ys")
                        rz = rz_sb[:, c * NST + st, :]
                        if st % 2 == 0:
                            nc.vector.tensor_scalar_mul(ys, y_ps, rz)
                        else:
                            nc.scalar.mul(ys, y_ps, rz)
                        nc.sync.dma_start(
                            y[c * 512 + st * P : c * 512 + (st + 1) * P, :], ys
                        )
                    return part2

                for st in range(NST):
                    deferred.append(make_part2(st))

            def epi_st(c, o_ps, st):
                # per-s-tile epilogue, fired as soon as O group st closes
                last = c == NB - 1
                if st == 0:
                    epi["o_sb"] = obp.tile([P, NST, P], BF16, tag="ob", name="o_sb")
                    epi["ot_sb"] = otbp.tile(
                        [P, NST, P], BF16, tag="otb", name="ot_sb"
                    )
                    if last:
                        epi["tr"] = trp.tile(
                            [HD, NST, P], BF16, tag="tr", name="tr"
                        )
                o_sb, ot_sb = epi["o_sb"], epi["ot_sb"]
                nc.vector.reciprocal(
                    rz_sb[:, c * NST + st, 0:1], o_ps[:, st, HD : HD + 1]
                )
                if st % 2 == 0:
                    nc.vector.tensor_copy(o_sb[:, st, :HD], o_ps[:, st, :HD])
                else:
                    nc.scalar.copy(o_sb[:, st, :HD], o_ps[:, st, :HD])
                if last:
                    tr_ps = epi["tr"]
                    nc.tensor.transpose(
                        tr_ps[:, st, :], o_sb[:, st, :HD], id_sb[:, :P]
                    )
                    if st % 2 == 0:
                        nc.vector.tensor_copy(ot_sb[:HD, st, :], tr_ps[:, st, :])
                    else:
                        nc.scalar.copy(ot_sb[:HD, st, :], tr_ps[:, st, :])
                else:
                    nc.sync.dma_start(
                        ot_sb[:, st, :], o_sb[:, st, :], transpose=True
                    )

                def part2():
                    if last:
                        y_ps = sp.tile([P, D], F32, tag="s_ps", name="y_ps_t")
                    else:
                        y_ps = yp.tile([P, D], F32, tag="y_ps")
                    nc.tensor.matmul(
                        y_ps, ot_sb[:HD, st, :], wp_sb,
                        start=True, stop=True,
                    )
                    ys = ysp.tile([P, D], BF16, tag="ys")
                    rz = rz_sb[:, c * NST + st, :]
                    if st % 2 == 0:
                        nc.vector.tensor_scalar_mul(ys, y_ps, rz)
                    else:
                        nc.scalar.mul(ys, y_ps, rz)
                    nc.sync.dma_start(
                        y[c * 512 + st * P : c * 512 + (st + 1) * P, :], ys
                    )

                deferred.append(part2)

            # slot machinery: O matmuls trail the scores by one full chunk
            # so the four o_ps accumulation groups run SEQUENTIALLY (psum
            # banks cannot hold two concurrently-open accumulation groups).
            state = {"nslot": 0, "prev": None, "cur": None, "o_ps": None}

            def start_chunk(c):
                state["c"] = c
                state["ssl"] = slice(c * 512, (c + 1) * 512)
                state["prev"] = state["cur"]
                state["cur"] = []

            def emit_o(t):
                # 4 trailing O matmuls for the previous chunk at local slot t
                pc, prev_e, o_ps = state["c"] - 1, state["prev"], state["o_ps"]
                st = t // 8
                for j in range(4):
                    tp = (t % 8) * 4 + j
                    nc.tensor.matmul(
                        o_ps[:, st, :],
                        prev_e[tp][:, st * P : (st + 1) * P],
                        v_sb[:, tp, :],
                        start=(tp == 0), stop=(tp == NT - 1),
                    )
                if pc == NB - 1:
                    if t % 8 == 7:
                        epi_st(pc, o_ps, t // 8)
                elif t == NT - 1:
                    state["epi_bulk"] = (pc, o_ps)

            def slot(t):
                c, ssl = state["c"], state["ssl"]
                s_ps = sp.tile([P, 512], F32, tag="s_ps")
                nc.tensor.matmul(
                    s_ps, kk[:, t * P : (t + 1) * P], qq[:, ssl],
                    start=True, stop=True,
                )
                if lanes[t] == "ACT":
                    e_sb = eap.tile([P, 512], BF16, tag="ea")
                    nc.scalar.activation(e_sb, s_ps, Exp)
                    e_bf = e_sb
                else:
                    e_sb = edp.tile([P, 512], I16, tag="ed")
                    nc.vector.tensor_scalar(e_sb, s_ps, EXP_L, EXP_C, Mult, Add)
                    e_bf = e_sb.bitcast(BF16)
                state["cur"].append(e_bf)
                if state["prev"] is not None:
                    if t == 0:
                        state["o_ps"] = op.tile(
                            [P, NST, HD + 1], F32, tag="o_ps", name="o_ps"
                        )
                    emit_o(t)
                state["nslot"] += 1
                if state.get("epi_bulk") and state["nslot"] % NT == bulkslot:
                    pc_, ops_ = state.pop("epi_bulk")
                    epi_bulk(pc_, ops_)
                if deferred and state["nslot"] % 8 == popslot:
                    deferred.pop(0)()

            def drain():
                # trailing O matmuls + epilogue for the final chunk
                state["prev"] = state["cur"]
                state["c"] += 1
                for t in range(NT):
                    if t == 0:
                        state["o_ps"] = op.tile(
                            [P, NST, HD + 1], F32, tag="o_ps", name="o_ps"
                        )
                        if state.get("epi_bulk"):
                            pc_, ops_ = state.pop("epi_bulk")
                            epi_bulk(pc_, ops_)
                    emit_o(t)
                    if deferred and t in ((10, 14, 18, 22, 26, 30), (12, 16, 20, 24, 28, 31), (8, 10, 12, 14, 16, 18), (14, 17, 20, 23, 26, 29))[dpops]:
                        deferred.pop(0)()

            # ---- phase B+C0: projection interleaved with chunk 0 ----
            start_chunk(0)
            for b in range(NB):
                ssl = slice(b * 512, (b + 1) * 512)
                qk_ps = sp.tile([P, 512], F32, tag="s_ps")
                for a in range(KT):
                    nc.tensor.matmul(
                        qk_ps, wqk_sb[:, a, :], xt_sb[:, a, ssl],
                        start=(a == 0), stop=(a == KT - 1),
                    )
                nc.scalar.copy(qq[:, ssl], qk_ps[:HD, :])
                nc.vector.tensor_copy(kk[:, ssl], qk_ps[HD:, :])
                v_ps = yp.tile([P, D], F32, tag="y_ps")
                for i in range(4):
                    t = b * 4 + i
                    tsl = slice(t * P, (t + 1) * P)
                    for a in range(KT):
                        nc.tensor.matmul(
                            v_ps[:, i * HD : (i + 1) * HD],
                            xt_sb[:, a, tsl], wv_sb[:, a, :],
                            start=(a == 0), stop=(a == KT - 1),
                        )
                v_ps_v = v_ps.rearrange("p (i d) -> p i d", d=HD)[:, :4, :]
                if b % 2 == 0:
                    nc.scalar.copy(v_sb[:, b * 4 : (b + 1) * 4, :HD], v_ps_v)
                else:
                    nc.vector.tensor_copy(v_sb[:, b * 4 : (b + 1) * 4, :HD], v_ps_v)
                # chunk-0 slots for the previous block's t-tiles
                if b >= 1:
                    for t in range((b - 1) * 4, b * 4):
                        slot(t)
            for t in range((NB - 1) * 4, NB * 4):
                slot(t)

            # ---- phase C: chunks 1..7 (same pipeline, no drain between) ----
            for c in range(1, NB):
                start_chunk(c)
                for t in range(NT):
                    slot(t)
            drain()
            while deferred:
                deferred.pop(0)()

    nc.compile()
    return nc


def run(inputs, trace=False, **build_kwargs):
    x = np.asarray(inputs["x"], dtype=np.float32)
    q_param = np.asarray(inputs["q_param"], dtype=np.float32)
    k_param = np.asarray(inputs["k_param"], dtype=np.float32)
    v_param = np.asarray(inputs["v_param"], dtype=np.float32)
    p_param = np.asarray(inputs["p_param"], dtype=np.float32)

    bf = ml_dtypes.bfloat16
    xt = np.ascontiguousarray(x[0].T).astype(bf)
    ident = np.tile(np.eye(P, dtype=np.float32), (1, 4)).astype(bf)
    in_maps = []
    for h in range(H):
        wqk = np.concatenate(
            [q_param[:, h, :] * SCALE, k_param[:, h, :]], axis=1
        )
        in_maps.append(
            {
                "xt": xt,
                "wqk": np.ascontiguousarray(wqk).astype(bf),
                "wv": np.ascontiguousarray(v_param[:, h, :]).astype(bf),
                "wp": np.ascontiguousarray(p_param[h]).astype(bf),
                "ident": ident,
            }
        )

    nc = build_kernel(**build_kwargs)
    res = run_bass_kernel_spmd(nc, in_maps, core_ids=list(range(H)), trace=trace)
    out = np.zeros((S, D), dtype=np.float32)
    for h in range(H):
        out += res.results[h]["y"].astype(np.float32)
    return out[None, :, :], res


def kernel(**inputs) -> np.ndarray:
    out, _ = run(inputs, trace=False)
    return out


# revision 9
# speedup vs baseline: 1.0228x; 1.0228x over previous
"""Multi-head attention (B=1, S=4096, D=512, H=8, HD=64) on 8 trn2 NeuronCores.

Sharding: one head per core. Each core projects Q/K/V for its head from a
bf16 copy of x^T, runs attention, applies its head's output projection, and
writes a full [S, D] fp32 partial; the host sums the 8 partials.

Structure (per core):
- All matmuls bf16 (fp32 PSUM accumulation). wq is pre-scaled by 1/sqrt(HD)
  on the host and packed with wk into one [D, 128] tensor so Q^T and K^T come
  out of one matmul stream ([128, 512] PSUM tiles: rows 0-63 Q^T, 64-127 K^T).
- Scores^T tiles [128 t, 512 s]: lhsT = K^T tile (stationary), rhs = Q^T.
- exp splits between ACT (exact, bf16 out) and DVE (Schraudolph: one
  tensor_scalar producing int16 bits that are the bf16 encoding of e^s).
- O accumulation is E-stationary: o_ps[s-tile, 65] += E_tile^T @ [V | 1];
  the softmax denominator Z lands on column 64, per-s on partitions.
- Epilogue per s-chunk: recip(Z), O -> bf16 (padded to 128 cols), DMA
  transpose to O^T, y_ps = O^T.T @ wp, scale by 1/Z into SBUF, DMA out.
  All deferred into the next chunk's instruction stream.
- Chunk 0 of the attention loop is interleaved with the projection phase
  (each score tile only needs K^T tiles already projected), hiding the
  x-load and projection behind chunk-0 lane work.
"""

import numpy as np
import ml_dtypes

import concourse.bacc as bacc
import concourse.mybir as mybir
import concourse.tile as tile
from concourse.bass_utils import run_bass_kernel_spmd

S = 4096
D = 512
HD = 64
H = 8
P = 128
KT = D // P            # 4 c-tiles
NB = S // 512          # 8 s-blocks / s-chunks
NT = S // P            # 32 t-tiles
NST = 512 // P         # 4 s-tiles per chunk
SCALE = HD ** -0.5

F32 = mybir.dt.float32
BF16 = mybir.dt.bfloat16
I16 = mybir.dt.int16

EXP_L = 128.0 / float(np.log(2.0))     # schraudolph multiplier for bf16 bits
EXP_C = 16256.0 - 5.5                  # bias (127<<7), centered


def build_kernel(n_dve=15, ebufs=37, n_warm=6, popslot=4, bulkslot=2, loff=0, dpops=0):
    """n_dve: DVE (approx-exp) t-tiles out of NT=32 per chunk."""
    nc = bacc.Bacc(
        "TRN2",
        target_bir_lowering=False,
        debug=False,
        enable_asserts=False,
        num_devices=H,
    )

    xt = nc.dram_tensor("xt", [D, S], BF16, kind="ExternalInput").ap()
    wqk = nc.dram_tensor("wqk", [D, P], BF16, kind="ExternalInput").ap()
    wv = nc.dram_tensor("wv", [D, HD], BF16, kind="ExternalInput").ap()
    wp = nc.dram_tensor("wp", [HD, D], BF16, kind="ExternalInput").ap()
    ident = nc.dram_tensor("ident", [P, 4 * P], BF16, kind="ExternalInput").ap()
    y = nc.dram_tensor("y", [S, D], BF16, kind="ExternalOutput").ap()

    Exp = mybir.ActivationFunctionType.Exp
    Mult = mybir.AluOpType.mult
    Add = mybir.AluOpType.add

    # lane pattern per chunk: n_dve DVE t-tiles spread among ACT t-tiles
    lanes = ["ACT"] * NT
    if n_dve > 0:
        step = NT / n_dve
        for i in range(n_dve):
            lanes[min(NT - 1, int(i * step) + loff)] = "DVE"

    with tile.TileContext(nc) as tc:
        with (
            tc.tile_pool(name="const", bufs=1) as cp,
            tc.tile_pool(name="ea", bufs=ebufs) as eap,
            tc.tile_pool(name="ed", bufs=ebufs) as edp,
            tc.tile_pool(name="ob", bufs=2) as obp,
            tc.tile_pool(name="otb", bufs=2) as otbp,
            tc.tile_pool(name="ys", bufs=3) as ysp,
            tc.tile_pool(name="xtp", bufs=1) as xtp,
            tc.tile_pool(name="sp", bufs=4, space="PSUM") as sp,
            tc.tile_pool(name="op", bufs=2, space="PSUM") as op,
            tc.tile_pool(name="yp", bufs=1, space="PSUM") as yp,
            tc.tile_pool(name="trp", bufs=1, space="PSUM") as trp,
        ):
            wqk_sb = cp.tile([P, KT, P], BF16, tag="wqk")
            id_sb = cp.tile([P, 4 * P], BF16, tag="ident")
            wv_sb = cp.tile([P, KT, HD], BF16, tag="wv")
            wp_sb = cp.tile([HD, D], BF16, tag="wp")
            qq = cp.tile([HD, S], BF16, tag="qq")      # Q^T (pre-scaled)
            kk = cp.tile([HD, S], BF16, tag="kk")      # K^T
            v_sb = cp.tile([P, NT, HD + 1], BF16, tag="v")
            rz_sb = cp.tile([P, NT, 1], F32, tag="rz")  # 1/Z per s-tile
            xt_sb = xtp.tile([P, KT, S], BF16, tag="xt")

            wsrc = cp.tile([P, 512], BF16, tag="wsrc")
            nc.gpsimd.memset(wsrc, 1.0)
            xt_r = xt.rearrange("(a p) s -> p a s", p=P)
            nc.sync.dma_start(xt_sb[:, :, 0:512], xt_r[:, :, 0:512])
            nc.sync.dma_start(wqk_sb, wqk.rearrange("(a p) d -> p a d", p=P))
            nc.sync.dma_start(id_sb, ident)
            nc.sync.dma_start(wv_sb, wv.rearrange("(a p) d -> p a d", p=P))
            for b in range(1, NB):
                ssl = slice(b * 512, (b + 1) * 512)
                nc.sync.dma_start(xt_sb[:, :, ssl], xt_r[:, :, ssl])
            nc.sync.dma_start(wp_sb, wp)
            nc.gpsimd.memset(v_sb[:, :, HD : HD + 1], 1.0)

            # PE warm-up: matmuls on a memset tile need no DMA, so the PE
            # clock ramp starts immediately while the first x block lands.
            for _ in range(n_warm):
                warm_ps = sp.tile([P, 512], F32, tag="s_ps", name="warm_ps")
                nc.tensor.matmul(
                    warm_ps, wsrc[:, :P], wsrc, start=True, stop=True
                )

            deferred = []
            epi = {}

            def epi_bulk(c, o_ps):
                o_sb = obp.tile([P, NST, P], BF16, tag="ob", name="o_sb")
                ot_sb = otbp.tile([P, NST, P], BF16, tag="otb", name="ot_sb")
                nc.vector.reciprocal(
                    rz_sb[:, c * NST : (c + 1) * NST, 0], o_ps[:, :, HD]
                )
                nc.vector.tensor_copy(o_sb[:, :, :HD], o_ps[:, :, :HD])
                for st in range(NST):
                    nc.sync.dma_start(
                        ot_sb[:, st, :], o_sb[:, st, :], transpose=True
                    )

                def make_part2(sp0):
                    def part2():
                        ys = ysp.tile([P, 2, D], BF16, tag="ys")
                        for i in range(2):
                            st = sp0 + i
                            y_ps = yp.tile([P, D], F32, tag="y_ps")
                            nc.tensor.matmul(
                                y_ps, ot_sb[:HD, st, :], wp_sb,
                                start=True, stop=True,
                            )
                            rz = rz_sb[:, c * NST + st, :]
                            if st % 2 == 0:
                                nc.vector.tensor_scalar_mul(ys[:, i, :], y_ps, rz)
                            else:
                                nc.scalar.mul(ys[:, i, :], y_ps, rz)
                        yd = y[c * 512 + sp0 * P : c * 512 + (sp0 + 2) * P, :]
                        nc.sync.dma_start(yd.rearrange("(a p) d -> p a d", p=P), ys)
                    return part2

                for sp0 in (0, 2):
                    deferred.append(make_part2(sp0))

            def epi_st(c, o_ps, st):
                # per-s-tile epilogue, fired as soon as O group st closes
                last = c == NB - 1
                if st == 0:
                    epi["o_sb"] = obp.tile([P, NST, P], BF16, tag="ob", name="o_sb")
                    epi["ot_sb"] = otbp.tile(
                        [P, NST, P], BF16, tag="otb", name="ot_sb"
                    )
                    if last:
                        epi["tr"] = trp.tile(
                            [HD, NST, P], BF16, tag="tr", name="tr"
                        )
                o_sb, ot_sb = epi["o_sb"], epi["ot_sb"]
                nc.vector.reciprocal(
                    rz_sb[:, c * NST + st, 0:1], o_ps[:, st, HD : HD + 1]
                )
                if st % 2 == 0:
                    nc.vector.tensor_copy(o_sb[:, st, :HD], o_ps[:, st, :HD])
                else:
                    nc.scalar.copy(o_sb[:, st, :HD], o_ps[:, st, :HD])
                if last:
                    tr_ps = epi["tr"]
                    nc.tensor.transpose(
                        tr_ps[:, st, :], o_sb[:, st, :HD], id_sb[:, :P]
                    )
                    if st % 2 == 0:
                        nc.vector.tensor_copy(ot_sb[:HD, st, :], tr_ps[:, st, :])
                    else:
                        nc.scalar.copy(ot_sb[:HD, st, :], tr_ps[:, st, :])
                else:
                    nc.sync.dma_start(
                        ot_sb[:, st, :], o_sb[:, st, :], transpose=True
                    )

                def part2():
                    sp0 = st - 1
                    ys = ysp.tile([P, 2, D], BF16, tag="ys")
                    split = last and st == NST - 1
                    for i in range(2):
                        sti = sp0 + i
                        if last:
                            y_ps = sp.tile([P, D], F32, tag="s_ps", name="y_ps_t")
                        else:
                            y_ps = yp.tile([P, D], F32, tag="y_ps")
                        nc.tensor.matmul(
                            y_ps, ot_sb[:HD, sti, :], wp_sb,
                            start=True, stop=True,
                        )
                        rz = rz_sb[:, c * NST + sti, :]
                        if sti % 2 == 0:
                            nc.vector.tensor_scalar_mul(ys[:, i, :], y_ps, rz)
                        else:
                            nc.scalar.mul(ys[:, i, :], y_ps, rz)
                        if split:
                            # issue each final write as soon as its scale is
                            # done: the very last transfer is half-size
                            nc.sync.dma_start(
                                y[c * 512 + sti * P : c * 512 + (sti + 1) * P, :],
                                ys[:, i, :],
                            )
                    if not split:
                        yd = y[c * 512 + sp0 * P : c * 512 + (sp0 + 2) * P, :]
                        nc.sync.dma_start(yd.rearrange("(a p) d -> p a d", p=P), ys)

                if st % 2 == 1:
                    deferred.append(part2)

            # slot machinery: O matmuls trail the scores by one full chunk
            # so the four o_ps accumulation groups run SEQUENTIALLY (psum
            # banks cannot hold two concurrently-open accumulation groups).
            state = {"nslot": 0, "prev": None, "cur": None, "o_ps": None}

            def start_chunk(c):
                state["c"] = c
                state["ssl"] = slice(c * 512, (c + 1) * 512)
                state["prev"] = state["cur"]
                state["cur"] = []

            def emit_o(t):
                # 4 trailing O matmuls for the previous chunk at local slot t
                pc, prev_e, o_ps = state["c"] - 1, state["prev"], state["o_ps"]
                st = t // 8
                for j in range(4):
                    tp = (t % 8) * 4 + j
                    nc.tensor.matmul(
                        o_ps[:, st, :],
                        prev_e[tp][:, st * P : (st + 1) * P],
                        v_sb[:, tp, :],
                        start=(tp == 0), stop=(tp == NT - 1),
                    )
                if pc == NB - 1:
                    if t % 8 == 7:
                        epi_st(pc, o_ps, t // 8)
                elif t == NT - 1:
                    state["epi_bulk"] = (pc, o_ps)

            def slot(t):
                c, ssl = state["c"], state["ssl"]
                s_ps = sp.tile([P, 512], F32, tag="s_ps")
                nc.tensor.matmul(
                    s_ps, kk[:, t * P : (t + 1) * P], qq[:, ssl],
                    start=True, stop=True,
                )
                if lanes[t] == "ACT":
                    e_sb = eap.tile([P, 512], BF16, tag="ea")
                    nc.scalar.activation(e_sb, s_ps, Exp)
                    e_bf = e_sb
                else:
                    e_sb = edp.tile([P, 512], I16, tag="ed")
                    nc.vector.tensor_scalar(e_sb, s_ps, EXP_L, EXP_C, Mult, Add)
                    e_bf = e_sb.bitcast(BF16)
                state["cur"].append(e_bf)
                if state["prev"] is not None:
                    if t == 0:
                        state["o_ps"] = op.tile(
                            [P, NST, HD + 1], F32, tag="o_ps", name="o_ps"
                        )
                    emit_o(t)
                state["nslot"] += 1
                if state.get("epi_bulk") and state["nslot"] % NT == bulkslot:
                    pc_, ops_ = state.pop("epi_bulk")
                    epi_bulk(pc_, ops_)
                if deferred and state["nslot"] % 8 == popslot:
                    deferred.pop(0)()

            def drain():
                # trailing O matmuls + epilogue for the final chunk
                state["prev"] = state["cur"]
                state["c"] += 1
                for t in range(NT):
                    if t == 0:
                        state["o_ps"] = op.tile(
                            [P, NST, HD + 1], F32, tag="o_ps", name="o_ps"
                        )
                        if state.get("epi_bulk"):
                            pc_, ops_ = state.pop("epi_bulk")
                            epi_bulk(pc_, ops_)
                    emit_o(t)
                    if deferred and t in ((10, 14, 18, 22, 26, 30), (12, 16, 20, 24, 28, 31), (8, 10, 12, 14, 16, 18), (14, 17, 20, 23, 26, 29))[dpops]:
                        deferred.pop(0)()

            # ---- phase B+C0: projection interleaved with chunk 0 ----
            start_chunk(0)
            for b in range(NB):
                ssl = slice(b * 512, (b + 1) * 512)
                qk_ps = sp.tile([P, 512], F32, tag="s_ps")
                for a in range(KT):
                    nc.tensor.matmul(
                        qk_ps, wqk_sb[:, a, :], xt_sb[:, a, ssl],
                        start=(a == 0), stop=(a == KT - 1),
                    )
                nc.scalar.copy(qq[:, ssl], qk_ps[:HD, :])
                nc.vector.tensor_copy(kk[:, ssl], qk_ps[HD:, :])
                v_ps = yp.tile([P, D], F32, tag="y_ps")
                for i in range(4):
                    t = b * 4 + i
                    tsl = slice(t * P, (t + 1) * P)
                    for a in range(KT):
                        nc.tensor.matmul(
                            v_ps[:, i * HD : (i + 1) * HD],
                            xt_sb[:, a, tsl], wv_sb[:, a, :],
                            start=(a == 0), stop=(a == KT - 1),
                        )
                v_ps_v = v_ps.rearrange("p (i d) -> p i d", d=HD)[:, :4, :]
                if b % 2 == 0:
                    nc.scalar.copy(v_sb[:, b * 4 : (b + 1) * 4, :HD], v_ps_v)
                else:
                    nc.vector.tensor_copy(v_sb[:, b * 4 : (b + 1) * 4, :HD], v_ps_v)
                # chunk-0 slots for the previous block's t-tiles
                if b >= 1:
                    for t in range((b - 1) * 4, b * 4):
                        slot(t)
            for t in range((NB - 1) * 4, NB * 4):
                slot(t)

            # ---- phase C: chunks 1..7 (same pipeline, no drain between) ----
            for c in range(1, NB):
                start_chunk(c)
                for t in range(NT):
                    slot(t)
            drain()
            while deferred:
                deferred.pop(0)()

    nc.compile()
    return nc


def run(inputs, trace=False, **build_kwargs):
    x = np.asarray(inputs["x"], dtype=np.float32)
    q_param = np.asarray(inputs["q_param"], dtype=np.float32)
    k_param = np.asarray(inputs["k_param"], dtype=np.float32)
    v_param = np.asarray(inputs["v_param"], dtype=np.float32)
    p_param = np.asarray(inputs["p_param"], dtype=np.float32)

    bf = ml_dtypes.bfloat16
    xt = np.ascontiguousarray(x[0].T).astype(bf)
    ident = np.tile(np.eye(P, dtype=np.float32), (1, 4)).astype(bf)
    in_maps = []
    for h in range(H):
        wqk = np.concatenate(
            [q_param[:, h, :] * SCALE, k_param[:, h, :]], axis=1
        )
        in_maps.append(
            {
                "xt": xt,
                "wqk": np.ascontiguousarray(wqk).astype(bf),
                "wv": np.ascontiguousarray(v_param[:, h, :]).astype(bf),
                "wp": np.ascontiguousarray(p_param[h]).astype(bf),
                "ident": ident,
            }
        )

    nc = build_kernel(**build_kwargs)
    res = run_bass_kernel_spmd(nc, in_maps, core_ids=list(range(H)), trace=trace)
    out = np.zeros((S, D), dtype=np.float32)
    for h in range(H):
        out += res.results[h]["y"].astype(np.float32)
    return out[None, :, :], res


def kernel(**inputs) -> np.ndarray:
    out, _ = run(inputs, trace=False)
    return out


# revision 10
# speedup vs baseline: 1.0232x; 1.0004x over previous
"""Multi-head attention (B=1, S=4096, D=512, H=8, HD=64) on 8 trn2 NeuronCores.

Sharding: one head per core. Each core projects Q/K/V for its head from a
bf16 copy of x^T, runs attention, applies its head's output projection, and
writes a full [S, D] fp32 partial; the host sums the 8 partials.

Structure (per core):
- All matmuls bf16 (fp32 PSUM accumulation). wq is pre-scaled by 1/sqrt(HD)
  on the host and packed with wk into one [D, 128] tensor so Q^T and K^T come
  out of one matmul stream ([128, 512] PSUM tiles: rows 0-63 Q^T, 64-127 K^T).
- Scores^T tiles [128 t, 512 s]: lhsT = K^T tile (stationary), rhs = Q^T.
- exp splits between ACT (exact, bf16 out) and DVE (Schraudolph: one
  tensor_scalar producing int16 bits that are the bf16 encoding of e^s).
- O accumulation is E-stationary: o_ps[s-tile, 65] += E_tile^T @ [V | 1];
  the softmax denominator Z lands on column 64, per-s on partitions.
- Epilogue per s-chunk: recip(Z), O -> bf16 (padded to 128 cols), DMA
  transpose to O^T, y_ps = O^T.T @ wp, scale by 1/Z into SBUF, DMA out.
  All deferred into the next chunk's instruction stream.
- Chunk 0 of the attention loop is interleaved with the projection phase
  (each score tile only needs K^T tiles already projected), hiding the
  x-load and projection behind chunk-0 lane work.
"""

import numpy as np
import ml_dtypes

import concourse.bacc as bacc
import concourse.mybir as mybir
import concourse.tile as tile
from concourse.bass_utils import run_bass_kernel_spmd

S = 4096
D = 512
HD = 64
H = 8
P = 128
KT = D // P            # 4 c-tiles
NB = S // 512          # 8 s-blocks / s-chunks
NT = S // P            # 32 t-tiles
NST = 512 // P         # 4 s-tiles per chunk
SCALE = HD ** -0.5

F32 = mybir.dt.float32
BF16 = mybir.dt.bfloat16
I16 = mybir.dt.int16

EXP_L = 128.0 / float(np.log(2.0))     # schraudolph multiplier for bf16 bits
EXP_C = 16256.0 - 5.5                  # bias (127<<7), centered


def build_kernel(n_dve=15, ebufs=37, n_warm=6, popslot=4, bulkslot=2, loff=0, dpops=0):
    """n_dve: DVE (approx-exp) t-tiles out of NT=32 per chunk."""
    nc = bacc.Bacc(
        "TRN2",
        target_bir_lowering=False,
        debug=False,
        enable_asserts=False,
        num_devices=H,
    )

    xt = nc.dram_tensor("xt", [D, S], BF16, kind="ExternalInput").ap()
    wqk = nc.dram_tensor("wqk", [D, P], BF16, kind="ExternalInput").ap()
    wv = nc.dram_tensor("wv", [D, HD], BF16, kind="ExternalInput").ap()
    wp = nc.dram_tensor("wp", [HD, D], BF16, kind="ExternalInput").ap()
    ident = nc.dram_tensor("ident", [P, 4 * P], BF16, kind="ExternalInput").ap()
    y = nc.dram_tensor("y", [S, D], BF16, kind="ExternalOutput").ap()

    Exp = mybir.ActivationFunctionType.Exp
    Mult = mybir.AluOpType.mult
    Add = mybir.AluOpType.add

    # lane pattern per chunk: n_dve DVE t-tiles spread among ACT t-tiles
    lanes = ["ACT"] * NT
    if n_dve > 0:
        step = NT / n_dve
        for i in range(n_dve):
            lanes[min(NT - 1, int(i * step) + loff)] = "DVE"

    with tile.TileContext(nc) as tc:
        with (
            tc.tile_pool(name="const", bufs=1) as cp,
            tc.tile_pool(name="ea", bufs=ebufs) as eap,
            tc.tile_pool(name="ed", bufs=ebufs) as edp,
            tc.tile_pool(name="ob", bufs=2) as obp,
            tc.tile_pool(name="otb", bufs=2) as otbp,
            tc.tile_pool(name="ys", bufs=3) as ysp,
            tc.tile_pool(name="xtp", bufs=1) as xtp,
            tc.tile_pool(name="sp", bufs=4, space="PSUM") as sp,
            tc.tile_pool(name="op", bufs=2, space="PSUM") as op,
            tc.tile_pool(name="yp", bufs=1, space="PSUM") as yp,
            tc.tile_pool(name="trp", bufs=1, space="PSUM") as trp,
        ):
            wqk_sb = cp.tile([P, KT, P], BF16, tag="wqk")
            id_sb = cp.tile([P, 4 * P], BF16, tag="ident")
            wv_sb = cp.tile([P, KT, HD], BF16, tag="wv")
            wp_sb = cp.tile([HD, D], BF16, tag="wp")
            qq = cp.tile([HD, S], BF16, tag="qq")      # Q^T (pre-scaled)
            kk = cp.tile([HD, S], BF16, tag="kk")      # K^T
            v_sb = cp.tile([P, NT, HD + 1], BF16, tag="v")
            rz_sb = cp.tile([P, NT, 1], F32, tag="rz")  # 1/Z per s-tile
            xt_sb = xtp.tile([P, KT, S], BF16, tag="xt")

            wsrc = cp.tile([P, 512], BF16, tag="wsrc")
            nc.gpsimd.memset(wsrc, 1.0)
            xt_r = xt.rearrange("(a p) s -> p a s", p=P)
            nc.sync.dma_start(xt_sb[:, :, 0:512], xt_r[:, :, 0:512])
            nc.sync.dma_start(wqk_sb, wqk.rearrange("(a p) d -> p a d", p=P))
            nc.sync.dma_start(id_sb, ident)
            nc.sync.dma_start(wv_sb, wv.rearrange("(a p) d -> p a d", p=P))
            for b in (1, 2, 3):
                ssl = slice(b * 512, (b + 1) * 512)
                nc.sync.dma_start(xt_sb[:, :, ssl], xt_r[:, :, ssl])
            for b in (4, 6):
                ssl = slice(b * 512, (b + 2) * 512)
                nc.sync.dma_start(xt_sb[:, :, ssl], xt_r[:, :, ssl])
            nc.sync.dma_start(wp_sb, wp)
            nc.gpsimd.memset(v_sb[:, :, HD : HD + 1], 1.0)

            # PE warm-up: matmuls on a memset tile need no DMA, so the PE
            # clock ramp starts immediately while the first x block lands.
            for _ in range(n_warm):
                warm_ps = sp.tile([P, 512], F32, tag="s_ps", name="warm_ps")
                nc.tensor.matmul(
                    warm_ps, wsrc[:, :P], wsrc, start=True, stop=True
                )

            deferred = []
            epi = {}

            def epi_bulk(c, o_ps):
                o_sb = obp.tile([P, NST, P], BF16, tag="ob", name="o_sb")
                ot_sb = otbp.tile([P, NST, P], BF16, tag="otb", name="ot_sb")
                nc.vector.reciprocal(
                    rz_sb[:, c * NST : (c + 1) * NST, 0], o_ps[:, :, HD]
                )
                nc.vector.tensor_copy(o_sb[:, :, :HD], o_ps[:, :, :HD])
                for st in range(NST):
                    nc.sync.dma_start(
                        ot_sb[:, st, :], o_sb[:, st, :], transpose=True
                    )

                def make_part2(sp0):
                    def part2():
                        ys = ysp.tile([P, 2, D], BF16, tag="ys")
                        for i in range(2):
                            st = sp0 + i
                            y_ps = yp.tile([P, D], F32, tag="y_ps")
                            nc.tensor.matmul(
                                y_ps, ot_sb[:HD, st, :], wp_sb,
                                start=True, stop=True,
                            )
                            rz = rz_sb[:, c * NST + st, :]
                            if st % 2 == 0:
                                nc.vector.tensor_scalar_mul(ys[:, i, :], y_ps, rz)
                            else:
                                nc.scalar.mul(ys[:, i, :], y_ps, rz)
                        yd = y[c * 512 + sp0 * P : c * 512 + (sp0 + 2) * P, :]
                        nc.sync.dma_start(yd.rearrange("(a p) d -> p a d", p=P), ys)
                    return part2

                for sp0 in (0, 2):
                    deferred.append(make_part2(sp0))

            def epi_st(c, o_ps, st):
                # per-s-tile epilogue, fired as soon as O group st closes
                last = c == NB - 1
                if st == 0:
                    epi["o_sb"] = obp.tile([P, NST, P], BF16, tag="ob", name="o_sb")
                    epi["ot_sb"] = otbp.tile(
                        [P, NST, P], BF16, tag="otb", name="ot_sb"
                    )
                    if last:
                        epi["tr"] = trp.tile(
                            [HD, NST, P], BF16, tag="tr", name="tr"
                        )
                o_sb, ot_sb = epi["o_sb"], epi["ot_sb"]
                nc.vector.reciprocal(
                    rz_sb[:, c * NST + st, 0:1], o_ps[:, st, HD : HD + 1]
                )
                if st % 2 == 0:
                    nc.vector.tensor_copy(o_sb[:, st, :HD], o_ps[:, st, :HD])
                else:
                    nc.scalar.copy(o_sb[:, st, :HD], o_ps[:, st, :HD])
                if last:
                    tr_ps = epi["tr"]
                    nc.tensor.transpose(
                        tr_ps[:, st, :], o_sb[:, st, :HD], id_sb[:, :P]
                    )
                    if st % 2 == 0:
                        nc.vector.tensor_copy(ot_sb[:HD, st, :], tr_ps[:, st, :])
                    else:
                        nc.scalar.copy(ot_sb[:HD, st, :], tr_ps[:, st, :])
                else:
                    nc.sync.dma_start(
                        ot_sb[:, st, :], o_sb[:, st, :], transpose=True
                    )

                def part2():
                    sp0 = st - 1
                    ys = ysp.tile([P, 2, D], BF16, tag="ys")
                    split = last and st == NST - 1
                    for i in range(2):
                        sti = sp0 + i
                        if last:
                            y_ps = sp.tile([P, D], F32, tag="s_ps", name="y_ps_t")
                        else:
                            y_ps = yp.tile([P, D], F32, tag="y_ps")
                        nc.tensor.matmul(
                            y_ps, ot_sb[:HD, sti, :], wp_sb,
                            start=True, stop=True,
                        )
                        rz = rz_sb[:, c * NST + sti, :]
                        if sti % 2 == 0:
                            nc.vector.tensor_scalar_mul(ys[:, i, :], y_ps, rz)
                        else:
                            nc.scalar.mul(ys[:, i, :], y_ps, rz)
                        if split:
                            # issue each final write as soon as its scale is
                            # done: the very last transfer is half-size
                            nc.sync.dma_start(
                                y[c * 512 + sti * P : c * 512 + (sti + 1) * P, :],
                                ys[:, i, :],
                            )
                    if not split:
                        yd = y[c * 512 + sp0 * P : c * 512 + (sp0 + 2) * P, :]
                        nc.sync.dma_start(yd.rearrange("(a p) d -> p a d", p=P), ys)

                if st % 2 == 1:
                    deferred.append(part2)

            # slot machinery: O matmuls trail the scores by one full chunk
            # so the four o_ps accumulation groups run SEQUENTIALLY (psum
            # banks cannot hold two concurrently-open accumulation groups).
            state = {"nslot": 0, "prev": None, "cur": None, "o_ps": None}

            def start_chunk(c):
                state["c"] = c
                state["ssl"] = slice(c * 512, (c + 1) * 512)
                state["prev"] = state["cur"]
                state["cur"] = []

            def emit_o(t):
                # 4 trailing O matmuls for the previous chunk at local slot t
                pc, prev_e, o_ps = state["c"] - 1, state["prev"], state["o_ps"]
                st = t // 8
                for j in range(4):
                    tp = (t % 8) * 4 + j
                    nc.tensor.matmul(
                        o_ps[:, st, :],
                        prev_e[tp][:, st * P : (st + 1) * P],
                        v_sb[:, tp, :],
                        start=(tp == 0), stop=(tp == NT - 1),
                    )
                if pc == NB - 1:
                    if t % 8 == 7:
                        epi_st(pc, o_ps, t // 8)
                elif t == NT - 1:
                    state["epi_bulk"] = (pc, o_ps)

            def slot(t):
                c, ssl = state["c"], state["ssl"]
                s_ps = sp.tile([P, 512], F32, tag="s_ps")
                nc.tensor.matmul(
                    s_ps, kk[:, t * P : (t + 1) * P], qq[:, ssl],
                    start=True, stop=True,
                )
                if lanes[t] == "ACT":
                    e_sb = eap.tile([P, 512], BF16, tag="ea")
                    nc.scalar.activation(e_sb, s_ps, Exp)
                    e_bf = e_sb
                else:
                    e_sb = edp.tile([P, 512], I16, tag="ed")
                    nc.vector.tensor_scalar(e_sb, s_ps, EXP_L, EXP_C, Mult, Add)
                    e_bf = e_sb.bitcast(BF16)
                state["cur"].append(e_bf)
                if state["prev"] is not None:
                    if t == 0:
                        state["o_ps"] = op.tile(
                            [P, NST, HD + 1], F32, tag="o_ps", name="o_ps"
                        )
                    emit_o(t)
                state["nslot"] += 1
                if state.get("epi_bulk") and state["nslot"] % NT == bulkslot:
                    pc_, ops_ = state.pop("epi_bulk")
                    epi_bulk(pc_, ops_)
                if deferred and state["nslot"] % 8 == popslot:
                    deferred.pop(0)()

            def drain():
                # trailing O matmuls + epilogue for the final chunk
                state["prev"] = state["cur"]
                state["c"] += 1
                for t in range(NT):
                    if t == 0:
                        state["o_ps"] = op.tile(
                            [P, NST, HD + 1], F32, tag="o_ps", name="o_ps"
                        )
                        if state.get("epi_bulk"):
                            pc_, ops_ = state.pop("epi_bulk")
                            epi_bulk(pc_, ops_)
                    emit_o(t)
                    if deferred and t in ((10, 14, 18, 22, 26, 30), (12, 16, 20, 24, 28, 31), (8, 10, 12, 14, 16, 18), (14, 17, 20, 23, 26, 29))[dpops]:
                        deferred.pop(0)()

            # ---- phase B+C0: projection interleaved with chunk 0 ----
            start_chunk(0)
            for b in range(NB):
                ssl = slice(b * 512, (b + 1) * 512)
                qk_ps = sp.tile([P, 512], F32, tag="s_ps")
                for a in range(KT):
                    nc.tensor.matmul(
                        qk_ps, wqk_sb[:, a, :], xt_sb[:, a, ssl],
                        start=(a == 0), stop=(a == KT - 1),
                    )
                nc.scalar.copy(qq[:, ssl], qk_ps[:HD, :])
                nc.vector.tensor_copy(kk[:, ssl], qk_ps[HD:, :])
                v_ps = yp.tile([P, D], F32, tag="y_ps")
                for i in range(4):
                    t = b * 4 + i
                    tsl = slice(t * P, (t + 1) * P)
                    for a in range(KT):
                        nc.tensor.matmul(
                            v_ps[:, i * HD : (i + 1) * HD],
                            xt_sb[:, a, tsl], wv_sb[:, a, :],
                            start=(a == 0), stop=(a == KT - 1),
                        )
                v_ps_v = v_ps.rearrange("p (i d) -> p i d", d=HD)[:, :4, :]
                if b % 2 == 0:
                    nc.scalar.copy(v_sb[:, b * 4 : (b + 1) * 4, :HD], v_ps_v)
                else:
                    nc.vector.tensor_copy(v_sb[:, b * 4 : (b + 1) * 4, :HD], v_ps_v)
                # chunk-0 slots for the previous block's t-tiles
                if b >= 1:
                    for t in range((b - 1) * 4, b * 4):
                        slot(t)
            for t in range((NB - 1) * 4, NB * 4):
                slot(t)

            # ---- phase C: chunks 1..7 (same pipeline, no drain between) ----
            for c in range(1, NB):
                start_chunk(c)
                for t in range(NT):
                    slot(t)
            drain()
            while deferred:
                deferred.pop(0)()

    nc.compile()
    return nc


def run(inputs, trace=False, **build_kwargs):
    x = np.asarray(inputs["x"], dtype=np.float32)
    q_param = np.asarray(inputs["q_param"], dtype=np.float32)
    k_param = np.asarray(inputs["k_param"], dtype=np.float32)
    v_param = np.asarray(inputs["v_param"], dtype=np.float32)
    p_param = np.asarray(inputs["p_param"], dtype=np.float32)

    bf = ml_dtypes.bfloat16
    xt = np.ascontiguousarray(x[0].T).astype(bf)
    ident = np.tile(np.eye(P, dtype=np.float32), (1, 4)).astype(bf)
    in_maps = []
    for h in range(H):
        wqk = np.concatenate(
            [q_param[:, h, :] * SCALE, k_param[:, h, :]], axis=1
        )
        in_maps.append(
            {
                "xt": xt,
                "wqk": np.ascontiguousarray(wqk).astype(bf),
                "wv": np.ascontiguousarray(v_param[:, h, :]).astype(bf),
                "wp": np.ascontiguousarray(p_param[h]).astype(bf),
                "ident": ident,
            }
        )

    nc = build_kernel(**build_kwargs)
    res = run_bass_kernel_spmd(nc, in_maps, core_ids=list(range(H)), trace=trace)
    out = np.zeros((S, D), dtype=np.float32)
    for h in range(H):
        out += res.results[h]["y"].astype(np.float32)
    return out[None, :, :], res


def kernel(**inputs) -> np.ndarray:
    out, _ = run(inputs, trace=False)
    return out


# revision 11
# speedup vs baseline: 1.0237x; 1.0004x over previous
"""Multi-head attention (B=1, S=4096, D=512, H=8, HD=64) on 8 trn2 NeuronCores.

Sharding: one head per core. Each core projects Q/K/V for its head from a
bf16 copy of x^T, runs attention, applies its head's output projection, and
writes a full [S, D] fp32 partial; the host sums the 8 partials.

Structure (per core):
- All matmuls bf16 (fp32 PSUM accumulation). wq is pre-scaled by 1/sqrt(HD)
  on the host and packed with wk into one [D, 128] tensor so Q^T and K^T come
  out of one matmul stream ([128, 512] PSUM tiles: rows 0-63 Q^T, 64-127 K^T).
- Scores^T tiles [128 t, 512 s]: lhsT = K^T tile (stationary), rhs = Q^T.
- exp splits between ACT (exact, bf16 out) and DVE (Schraudolph: one
  tensor_scalar producing int16 bits that are the bf16 encoding of e^s).
- O accumulation is E-stationary: o_ps[s-tile, 65] += E_tile^T @ [V | 1];
  the softmax denominator Z lands on column 64, per-s on partitions.
- Epilogue per s-chunk: recip(Z), O -> bf16 (padded to 128 cols), DMA
  transpose to O^T, y_ps = O^T.T @ wp, scale by 1/Z into SBUF, DMA out.
  All deferred into the next chunk's instruction stream.
- Chunk 0 of the attention loop is interleaved with the projection phase
  (each score tile only needs K^T tiles already projected), hiding the
  x-load and projection behind chunk-0 lane work.
"""

import numpy as np
import ml_dtypes

import concourse.bacc as bacc
import concourse.mybir as mybir
import concourse.tile as tile
from concourse.bass_utils import run_bass_kernel_spmd

S = 4096
D = 512
HD = 64
H = 8
P = 128
KT = D // P            # 4 c-tiles
NB = S // 512          # 8 s-blocks / s-chunks
NT = S // P            # 32 t-tiles
NST = 512 // P         # 4 s-tiles per chunk
SCALE = HD ** -0.5

F32 = mybir.dt.float32
BF16 = mybir.dt.bfloat16
I16 = mybir.dt.int16

EXP_L = 128.0 / float(np.log(2.0))     # schraudolph multiplier for bf16 bits
EXP_C = 16256.0 - 5.5                  # bias (127<<7), centered


def build_kernel(n_dve=15, ebufs=37, n_warm=6, popslot=4, bulkslot=2, loff=0, dpops=0):
    """n_dve: DVE (approx-exp) t-tiles out of NT=32 per chunk."""
    nc = bacc.Bacc(
        "TRN2",
        target_bir_lowering=False,
        debug=False,
        enable_asserts=False,
        num_devices=H,
    )

    xt = nc.dram_tensor("xt", [D, S], BF16, kind="ExternalInput").ap()
    wqk = nc.dram_tensor("wqk", [D, P], BF16, kind="ExternalInput").ap()
    wv = nc.dram_tensor("wv", [D, HD], BF16, kind="ExternalInput").ap()
    wp = nc.dram_tensor("wp", [HD, D], BF16, kind="ExternalInput").ap()
    ident = nc.dram_tensor("ident", [P, 4 * P], BF16, kind="ExternalInput").ap()
    y = nc.dram_tensor("y", [S, D], BF16, kind="ExternalOutput").ap()

    Exp = mybir.ActivationFunctionType.Exp
    Mult = mybir.AluOpType.mult
    Add = mybir.AluOpType.add

    # lane pattern per chunk: n_dve DVE t-tiles spread among ACT t-tiles
    lanes = ["ACT"] * NT
    if n_dve > 0:
        step = NT / n_dve
        for i in range(n_dve):
            lanes[min(NT - 1, int(i * step) + loff)] = "DVE"

    with tile.TileContext(nc) as tc:
        with (
            tc.tile_pool(name="const", bufs=1) as cp,
            tc.tile_pool(name="ea", bufs=ebufs) as eap,
            tc.tile_pool(name="ed", bufs=ebufs) as edp,
            tc.tile_pool(name="ob", bufs=3) as obp,
            tc.tile_pool(name="otb", bufs=3) as otbp,
            tc.tile_pool(name="ys", bufs=3) as ysp,
            tc.tile_pool(name="xtp", bufs=1) as xtp,
            tc.tile_pool(name="sp", bufs=4, space="PSUM") as sp,
            tc.tile_pool(name="op", bufs=2, space="PSUM") as op,
            tc.tile_pool(name="yp", bufs=1, space="PSUM") as yp,
            tc.tile_pool(name="trp", bufs=1, space="PSUM") as trp,
        ):
            wqk_sb = cp.tile([P, KT, P], BF16, tag="wqk")
            id_sb = cp.tile([P, 4 * P], BF16, tag="ident")
            wv_sb = cp.tile([P, KT, HD], BF16, tag="wv")
            wp_sb = cp.tile([HD, D], BF16, tag="wp")
            qq = cp.tile([HD, S], BF16, tag="qq")      # Q^T (pre-scaled)
            kk = cp.tile([HD, S], BF16, tag="kk")      # K^T
            v_sb = cp.tile([P, NT, HD + 1], BF16, tag="v")
            rz_sb = cp.tile([P, NT, 1], F32, tag="rz")  # 1/Z per s-tile
            xt_sb = xtp.tile([P, KT, S], BF16, tag="xt")

            wsrc = cp.tile([P, 512], BF16, tag="wsrc")
            nc.gpsimd.memset(wsrc, 1.0)
            xt_r = xt.rearrange("(a p) s -> p a s", p=P)
            nc.sync.dma_start(xt_sb[:, :, 0:512], xt_r[:, :, 0:512])
            nc.sync.dma_start(wqk_sb, wqk.rearrange("(a p) d -> p a d", p=P))
            nc.sync.dma_start(id_sb, ident)
            nc.sync.dma_start(wv_sb, wv.rearrange("(a p) d -> p a d", p=P))
            for b in (1, 2, 3):
                ssl = slice(b * 512, (b + 1) * 512)
                nc.sync.dma_start(xt_sb[:, :, ssl], xt_r[:, :, ssl])
            for b in (4, 6):
                ssl = slice(b * 512, (b + 2) * 512)
                nc.sync.dma_start(xt_sb[:, :, ssl], xt_r[:, :, ssl])
            nc.sync.dma_start(wp_sb, wp)
            nc.gpsimd.memset(v_sb[:, :, HD : HD + 1], 1.0)

            # PE warm-up: matmuls on a memset tile need no DMA, so the PE
            # clock ramp starts immediately while the first x block lands.
            for _ in range(n_warm):
                warm_ps = sp.tile([P, 512], F32, tag="s_ps", name="warm_ps")
                nc.tensor.matmul(
                    warm_ps, wsrc[:, :P], wsrc, start=True, stop=True
                )

            deferred = []
            epi = {}

            def epi_bulk(c, o_ps):
                o_sb = obp.tile([P, NST, P], BF16, tag="ob", name="o_sb")
                ot_sb = otbp.tile([P, NST, P], BF16, tag="otb", name="ot_sb")
                nc.vector.reciprocal(
                    rz_sb[:, c * NST : (c + 1) * NST, 0], o_ps[:, :, HD]
                )
                nc.vector.tensor_copy(o_sb[:, :, :HD], o_ps[:, :, :HD])
                for st in range(NST):
                    nc.sync.dma_start(
                        ot_sb[:, st, :], o_sb[:, st, :], transpose=True
                    )

                def make_part2(sp0):
                    def part2():
                        ys = ysp.tile([P, 2, D], BF16, tag="ys")
                        for i in range(2):
                            st = sp0 + i
                            y_ps = yp.tile([P, D], F32, tag="y_ps")
                            nc.tensor.matmul(
                                y_ps, ot_sb[:HD, st, :], wp_sb,
                                start=True, stop=True,
                            )
                            rz = rz_sb[:, c * NST + st, :]
                            if st % 2 == 0:
                                nc.vector.tensor_scalar_mul(ys[:, i, :], y_ps, rz)
                            else:
                                nc.scalar.mul(ys[:, i, :], y_ps, rz)
                        yd = y[c * 512 + sp0 * P : c * 512 + (sp0 + 2) * P, :]
                        nc.sync.dma_start(yd.rearrange("(a p) d -> p a d", p=P), ys)
                    return part2

                for sp0 in (0, 2):
                    deferred.append(make_part2(sp0))

            def epi_st(c, o_ps, st):
                # per-s-tile epilogue, fired as soon as O group st closes
                last = c == NB - 1
                if st == 0:
                    epi["o_sb"] = obp.tile([P, NST, P], BF16, tag="ob", name="o_sb")
                    epi["ot_sb"] = otbp.tile(
                        [P, NST, P], BF16, tag="otb", name="ot_sb"
                    )
                    if last:
                        epi["tr"] = trp.tile(
                            [HD, NST, P], BF16, tag="tr", name="tr"
                        )
                o_sb, ot_sb = epi["o_sb"], epi["ot_sb"]
                nc.vector.reciprocal(
                    rz_sb[:, c * NST + st, 0:1], o_ps[:, st, HD : HD + 1]
                )
                if st % 2 == 0:
                    nc.vector.tensor_copy(o_sb[:, st, :HD], o_ps[:, st, :HD])
                else:
                    nc.scalar.copy(o_sb[:, st, :HD], o_ps[:, st, :HD])
                if last:
                    tr_ps = epi["tr"]
                    nc.tensor.transpose(
                        tr_ps[:, st, :], o_sb[:, st, :HD], id_sb[:, :P]
                    )
                    if st % 2 == 0:
                        nc.vector.tensor_copy(ot_sb[:HD, st, :], tr_ps[:, st, :])
                    else:
                        nc.scalar.copy(ot_sb[:HD, st, :], tr_ps[:, st, :])
                else:
                    nc.sync.dma_start(
                        ot_sb[:, st, :], o_sb[:, st, :], transpose=True
                    )

                def part2():
                    sp0 = st - 1
                    ys = ysp.tile([P, 2, D], BF16, tag="ys")
                    split = last and st == NST - 1
                    for i in range(2):
                        sti = sp0 + i
                        if last:
                            y_ps = sp.tile([P, D], F32, tag="s_ps", name="y_ps_t")
                        else:
                            y_ps = yp.tile([P, D], F32, tag="y_ps")
                        nc.tensor.matmul(
                            y_ps, ot_sb[:HD, sti, :], wp_sb,
                            start=True, stop=True,
                        )
                        rz = rz_sb[:, c * NST + sti, :]
                        if sti % 2 == 0:
                            nc.vector.tensor_scalar_mul(ys[:, i, :], y_ps, rz)
                        else:
                            nc.scalar.mul(ys[:, i, :], y_ps, rz)
                        if split:
                            # issue each final write as soon as its scale is
                            # done: the very last transfer is half-size
                            nc.sync.dma_start(
                                y[c * 512 + sti * P : c * 512 + (sti + 1) * P, :],
                                ys[:, i, :],
                            )
                    if not split:
                        yd = y[c * 512 + sp0 * P : c * 512 + (sp0 + 2) * P, :]
                        nc.sync.dma_start(yd.rearrange("(a p) d -> p a d", p=P), ys)

                if st % 2 == 1:
                    deferred.append(part2)

            # slot machinery: O matmuls trail the scores by one full chunk
            # so the four o_ps accumulation groups run SEQUENTIALLY (psum
            # banks cannot hold two concurrently-open accumulation groups).
            state = {"nslot": 0, "prev": None, "cur": None, "o_ps": None}

            def start_chunk(c):
                state["c"] = c
                state["ssl"] = slice(c * 512, (c + 1) * 512)
                state["prev"] = state["cur"]
                state["cur"] = []

            def emit_o(t):
                # 4 trailing O matmuls for the previous chunk at local slot t
                pc, prev_e, o_ps = state["c"] - 1, state["prev"], state["o_ps"]
                st = t // 8
                for j in range(4):
                    tp = (t % 8) * 4 + j
                    nc.tensor.matmul(
                        o_ps[:, st, :],
                        prev_e[tp][:, st * P : (st + 1) * P],
                        v_sb[:, tp, :],
                        start=(tp == 0), stop=(tp == NT - 1),
                    )
                if pc == NB - 1:
                    if t % 8 == 7:
                        epi_st(pc, o_ps, t // 8)
                elif t == NT - 1:
                    state["epi_bulk"] = (pc, o_ps)

            def slot(t):
                c, ssl = state["c"], state["ssl"]
                s_ps = sp.tile([P, 512], F32, tag="s_ps")
                nc.tensor.matmul(
                    s_ps, kk[:, t * P : (t + 1) * P], qq[:, ssl],
                    start=True, stop=True,
                )
                if lanes[t] == "ACT":
                    e_sb = eap.tile([P, 512], BF16, tag="ea")
                    nc.scalar.activation(e_sb, s_ps, Exp)
                    e_bf = e_sb
                else:
                    e_sb = edp.tile([P, 512], I16, tag="ed")
                    nc.vector.tensor_scalar(e_sb, s_ps, EXP_L, EXP_C, Mult, Add)
                    e_bf = e_sb.bitcast(BF16)
                state["cur"].append(e_bf)
                if state["prev"] is not None:
                    if t == 0:
                        state["o_ps"] = op.tile(
                            [P, NST, HD + 1], F32, tag="o_ps", name="o_ps"
                        )
                    emit_o(t)
                state["nslot"] += 1
                if state.get("epi_bulk") and state["nslot"] % NT == bulkslot:
                    pc_, ops_ = state.pop("epi_bulk")
                    epi_bulk(pc_, ops_)
                if deferred and state["nslot"] % 8 == popslot:
                    deferred.pop(0)()

            def drain():
                # trailing O matmuls + epilogue for the final chunk
                state["prev"] = state["cur"]
                state["c"] += 1
                for t in range(NT):
                    if t == 0:
                        state["o_ps"] = op.tile(
                            [P, NST, HD + 1], F32, tag="o_ps", name="o_ps"
                        )
                        if state.get("epi_bulk"):
                            pc_, ops_ = state.pop("epi_bulk")
                            epi_bulk(pc_, ops_)
                    emit_o(t)
                    if deferred and t in ((10, 14, 18, 22, 26, 30), (12, 16, 20, 24, 28, 31), (8, 10, 12, 14, 16, 18), (14, 17, 20, 23, 26, 29))[dpops]:
                        deferred.pop(0)()

            # ---- phase B+C0: projection interleaved with chunk 0 ----
            start_chunk(0)
            for b in range(NB):
                ssl = slice(b * 512, (b + 1) * 512)
                qk_ps = sp.tile([P, 512], F32, tag="s_ps")
                for a in range(KT):
                    nc.tensor.matmul(
                        qk_ps, wqk_sb[:, a, :], xt_sb[:, a, ssl],
                        start=(a == 0), stop=(a == KT - 1),
                    )
                nc.scalar.copy(qq[:, ssl], qk_ps[:HD, :])
                nc.vector.tensor_copy(kk[:, ssl], qk_ps[HD:, :])
                v_ps = yp.tile([P, D], F32, tag="y_ps")
                for i in range(4):
                    t = b * 4 + i
                    tsl = slice(t * P, (t + 1) * P)
                    for a in range(KT):
                        nc.tensor.matmul(
                            v_ps[:, i * HD : (i + 1) * HD],
                            xt_sb[:, a, tsl], wv_sb[:, a, :],
                            start=(a == 0), stop=(a == KT - 1),
                        )
                v_ps_v = v_ps.rearrange("p (i d) -> p i d", d=HD)[:, :4, :]
                if b % 2 == 0:
                    nc.scalar.copy(v_sb[:, b * 4 : (b + 1) * 4, :HD], v_ps_v)
                else:
                    nc.vector.tensor_copy(v_sb[:, b * 4 : (b + 1) * 4, :HD], v_ps_v)
                # chunk-0 slots for the previous block's t-tiles
                if b >= 1:
                    for t in range((b - 1) * 4, b * 4):
                        slot(t)
            for t in range((NB - 1) * 4, NB * 4):
                slot(t)

            # ---- phase C: chunks 1..7 (same pipeline, no drain between) ----
            for c in range(1, NB):
                start_chunk(c)
                for t in range(NT):
                    slot(t)
            drain()
            while deferred:
                deferred.pop(0)()

    nc.compile()
    return nc


def run(inputs, trace=False, **build_kwargs):
    x = np.asarray(inputs["x"], dtype=np.float32)
    q_param = np.asarray(inputs["q_param"], dtype=np.float32)
    k_param = np.asarray(inputs["k_param"], dtype=np.float32)
    v_param = np.asarray(inputs["v_param"], dtype=np.float32)
    p_param = np.asarray(inputs["p_param"], dtype=np.float32)

    bf = ml_dtypes.bfloat16
    xt = np.ascontiguousarray(x[0].T).astype(bf)
    ident = np.tile(np.eye(P, dtype=np.float32), (1, 4)).astype(bf)
    in_maps = []
    for h in range(H):
        wqk = np.concatenate(
            [q_param[:, h, :] * SCALE, k_param[:, h, :]], axis=1
        )
        in_maps.append(
            {
                "xt": xt,
                "wqk": np.ascontiguousarray(wqk).astype(bf),
                "wv": np.ascontiguousarray(v_param[:, h, :]).astype(bf),
                "wp": np.ascontiguousarray(p_param[h]).astype(bf),
                "ident": ident,
            }
        )

    nc = build_kernel(**build_kwargs)
    res = run_bass_kernel_spmd(nc, in_maps, core_ids=list(range(H)), trace=trace)
    out = np.zeros((S, D), dtype=np.float32)
    for h in range(H):
        out += res.results[h]["y"].astype(np.float32)
    return out[None, :, :], res


def kernel(**inputs) -> np.ndarray:
    out, _ = run(inputs, trace=False)
    return out


# revision 12
# speedup vs baseline: 1.0241x; 1.0004x over previous
"""Multi-head attention (B=1, S=4096, D=512, H=8, HD=64) on 8 trn2 NeuronCores.

Sharding: one head per core. Each core projects Q/K/V for its head from a
bf16 copy of x^T, runs attention, applies its head's output projection, and
writes a full [S, D] fp32 partial; the host sums the 8 partials.

Structure (per core):
- All matmuls bf16 (fp32 PSUM accumulation). wq is pre-scaled by 1/sqrt(HD)
  on the host and packed with wk into one [D, 128] tensor so Q^T and K^T come
  out of one matmul stream ([128, 512] PSUM tiles: rows 0-63 Q^T, 64-127 K^T).
- Scores^T tiles [128 t, 512 s]: lhsT = K^T tile (stationary), rhs = Q^T.
- exp splits between ACT (exact, bf16 out) and DVE (Schraudolph: one
  tensor_scalar producing int16 bits that are the bf16 encoding of e^s).
- O accumulation is E-stationary: o_ps[s-tile, 65] += E_tile^T @ [V | 1];
  the softmax denominator Z lands on column 64, per-s on partitions.
- Epilogue per s-chunk: recip(Z), O -> bf16 (padded to 128 cols), DMA
  transpose to O^T, y_ps = O^T.T @ wp, scale by 1/Z into SBUF, DMA out.
  All deferred into the next chunk's instruction stream.
- Chunk 0 of the attention loop is interleaved with the projection phase
  (each score tile only needs K^T tiles already projected), hiding the
  x-load and projection behind chunk-0 lane work.
"""

import numpy as np
import ml_dtypes

import concourse.bacc as bacc
import concourse.mybir as mybir
import concourse.tile as tile
from concourse.bass_utils import run_bass_kernel_spmd

S = 4096
D = 512
HD = 64
H = 8
P = 128
KT = D // P            # 4 c-tiles
NB = S // 512          # 8 s-blocks / s-chunks
NT = S // P            # 32 t-tiles
NST = 512 // P         # 4 s-tiles per chunk
SCALE = HD ** -0.5

F32 = mybir.dt.float32
BF16 = mybir.dt.bfloat16
I16 = mybir.dt.int16

EXP_L = 128.0 / float(np.log(2.0))     # schraudolph multiplier for bf16 bits
EXP_C = 16256.0 - 5.5                  # bias (127<<7), centered


def build_kernel(n_dve=15, ebufs=37, n_warm=6, popslot=4, bulkslot=2, loff=0, dpops=0):
    """n_dve: DVE (approx-exp) t-tiles out of NT=32 per chunk."""
    nc = bacc.Bacc(
        "TRN2",
        target_bir_lowering=False,
        debug=False,
        enable_asserts=False,
        num_devices=H,
    )

    xt = nc.dram_tensor("xt", [D, S], BF16, kind="ExternalInput").ap()
    wqk = nc.dram_tensor("wqk", [D, P], BF16, kind="ExternalInput").ap()
    wv = nc.dram_tensor("wv", [D, HD], BF16, kind="ExternalInput").ap()
    wp = nc.dram_tensor("wp", [HD, D], BF16, kind="ExternalInput").ap()
    ident = nc.dram_tensor("ident", [P, 4 * P], BF16, kind="ExternalInput").ap()
    y = nc.dram_tensor("y", [S, D], BF16, kind="ExternalOutput").ap()

    Exp = mybir.ActivationFunctionType.Exp
    Mult = mybir.AluOpType.mult
    Add = mybir.AluOpType.add

    # lane pattern per chunk: n_dve DVE t-tiles spread among ACT t-tiles
    lanes = ["ACT"] * NT
    if n_dve > 0:
        step = NT / n_dve
        for i in range(n_dve):
            lanes[min(NT - 1, int(i * step) + loff)] = "DVE"

    with tile.TileContext(nc) as tc:
        with (
            tc.tile_pool(name="const", bufs=1) as cp,
            tc.tile_pool(name="ea", bufs=ebufs) as eap,
            tc.tile_pool(name="ed", bufs=ebufs) as edp,
            tc.tile_pool(name="ob", bufs=4) as obp,
            tc.tile_pool(name="otb", bufs=4) as otbp,
            tc.tile_pool(name="ys", bufs=3) as ysp,
            tc.tile_pool(name="xtp", bufs=1) as xtp,
            tc.tile_pool(name="sp", bufs=4, space="PSUM") as sp,
            tc.tile_pool(name="op", bufs=2, space="PSUM") as op,
            tc.tile_pool(name="yp", bufs=1, space="PSUM") as yp,
            tc.tile_pool(name="trp", bufs=1, space="PSUM") as trp,
        ):
            wqk_sb = cp.tile([P, KT, P], BF16, tag="wqk")
            id_sb = cp.tile([P, 4 * P], BF16, tag="ident")
            wv_sb = cp.tile([P, KT, HD], BF16, tag="wv")
            wp_sb = cp.tile([HD, D], BF16, tag="wp")
            qq = cp.tile([HD, S], BF16, tag="qq")      # Q^T (pre-scaled)
            kk = cp.tile([HD, S], BF16, tag="kk")      # K^T
            v_sb = cp.tile([P, NT, HD + 1], BF16, tag="v")
            rz_sb = cp.tile([P, NT, 1], F32, tag="rz")  # 1/Z per s-tile
            xt_sb = xtp.tile([P, KT, S], BF16, tag="xt")

            wsrc = cp.tile([P, 512], BF16, tag="wsrc")
            nc.gpsimd.memset(wsrc, 1.0)
            xt_r = xt.rearrange("(a p) s -> p a s", p=P)
            nc.sync.dma_start(xt_sb[:, :, 0:512], xt_r[:, :, 0:512])
            nc.sync.dma_start(wqk_sb, wqk.rearrange("(a p) d -> p a d", p=P))
            nc.sync.dma_start(id_sb, ident)
            nc.sync.dma_start(wv_sb, wv.rearrange("(a p) d -> p a d", p=P))
            for b in (1, 2, 3):
                ssl = slice(b * 512, (b + 1) * 512)
                nc.sync.dma_start(xt_sb[:, :, ssl], xt_r[:, :, ssl])
            for b in (4, 6):
                ssl = slice(b * 512, (b + 2) * 512)
                nc.sync.dma_start(xt_sb[:, :, ssl], xt_r[:, :, ssl])
            nc.sync.dma_start(wp_sb, wp)
            nc.gpsimd.memset(v_sb[:, :, HD : HD + 1], 1.0)

            # PE warm-up: matmuls on a memset tile need no DMA, so the PE
            # clock ramp starts immediately while the first x block lands.
            for _ in range(n_warm):
                warm_ps = sp.tile([P, 512], F32, tag="s_ps", name="warm_ps")
                nc.tensor.matmul(
                    warm_ps, wsrc[:, :P], wsrc, start=True, stop=True
                )

            deferred = []
            epi = {}

            def epi_bulk(c, o_ps):
                o_sb = obp.tile([P, NST, P], BF16, tag="ob", name="o_sb")
                ot_sb = otbp.tile([P, NST, P], BF16, tag="otb", name="ot_sb")
                nc.vector.reciprocal(
                    rz_sb[:, c * NST : (c + 1) * NST, 0], o_ps[:, :, HD]
                )
                nc.vector.tensor_copy(o_sb[:, :, :HD], o_ps[:, :, :HD])
                for st in range(NST):
                    nc.sync.dma_start(
                        ot_sb[:, st, :], o_sb[:, st, :], transpose=True
                    )

                def make_part2(sp0):
                    def part2():
                        ys = ysp.tile([P, 2, D], BF16, tag="ys")
                        for i in range(2):
                            st = sp0 + i
                            y_ps = yp.tile([P, D], F32, tag="y_ps")
                            nc.tensor.matmul(
                                y_ps, ot_sb[:HD, st, :], wp_sb,
                                start=True, stop=True,
                            )
                            rz = rz_sb[:, c * NST + st, :]
                            if st % 2 == 0:
                                nc.vector.tensor_scalar_mul(ys[:, i, :], y_ps, rz)
                            else:
                                nc.scalar.mul(ys[:, i, :], y_ps, rz)
                        yd = y[c * 512 + sp0 * P : c * 512 + (sp0 + 2) * P, :]
                        nc.sync.dma_start(yd.rearrange("(a p) d -> p a d", p=P), ys)
                    return part2

                for sp0 in (0, 2):
                    deferred.append(make_part2(sp0))

            def epi_st(c, o_ps, st):
                # per-s-tile epilogue, fired as soon as O group st closes
                last = c == NB - 1
                if st == 0:
                    epi["o_sb"] = obp.tile([P, NST, P], BF16, tag="ob", name="o_sb")
                    epi["ot_sb"] = otbp.tile(
                        [P, NST, P], BF16, tag="otb", name="ot_sb"
                    )
                    if last:
                        epi["tr"] = trp.tile(
                            [HD, NST, P], BF16, tag="tr", name="tr"
                        )
                o_sb, ot_sb = epi["o_sb"], epi["ot_sb"]
                nc.vector.reciprocal(
                    rz_sb[:, c * NST + st, 0:1], o_ps[:, st, HD : HD + 1]
                )
                if st % 2 == 0:
                    nc.vector.tensor_copy(o_sb[:, st, :HD], o_ps[:, st, :HD])
                else:
                    nc.scalar.copy(o_sb[:, st, :HD], o_ps[:, st, :HD])
                if last:
                    tr_ps = epi["tr"]
                    nc.tensor.transpose(
                        tr_ps[:, st, :], o_sb[:, st, :HD], id_sb[:, :P]
                    )
                    if st % 2 == 0:
                        nc.vector.tensor_copy(ot_sb[:HD, st, :], tr_ps[:, st, :])
                    else:
                        nc.scalar.copy(ot_sb[:HD, st, :], tr_ps[:, st, :])
                else:
                    nc.sync.dma_start(
                        ot_sb[:, st, :], o_sb[:, st, :], transpose=True
                    )

                def part2():
                    sp0 = st - 1
                    ys = ysp.tile([P, 2, D], BF16, tag="ys")
                    split = last and st == NST - 1
                    for i in range(2):
                        sti = sp0 + i
                        if last:
                            y_ps = sp.tile([P, D], F32, tag="s_ps", name="y_ps_t")
                        else:
                            y_ps = yp.tile([P, D], F32, tag="y_ps")
                        nc.tensor.matmul(
                            y_ps, ot_sb[:HD, sti, :], wp_sb,
                            start=True, stop=True,
                        )
                        rz = rz_sb[:, c * NST + sti, :]
                        if sti % 2 == 0:
                            nc.vector.tensor_scalar_mul(ys[:, i, :], y_ps, rz)
                        else:
                            nc.scalar.mul(ys[:, i, :], y_ps, rz)
                        if split:
                            # issue each final write as soon as its scale is
                            # done: the very last transfer is half-size
                            nc.sync.dma_start(
                                y[c * 512 + sti * P : c * 512 + (sti + 1) * P, :],
                                ys[:, i, :],
                            )
                    if not split:
                        yd = y[c * 512 + sp0 * P : c * 512 + (sp0 + 2) * P, :]
                        nc.sync.dma_start(yd.rearrange("(a p) d -> p a d", p=P), ys)

                if st % 2 == 1:
                    deferred.append(part2)

            # slot machinery: O matmuls trail the scores by one full chunk
            # so the four o_ps accumulation groups run SEQUENTIALLY (psum
            # banks cannot hold two concurrently-open accumulation groups).
            state = {"nslot": 0, "prev": None, "cur": None, "o_ps": None}

            def start_chunk(c):
                state["c"] = c
                state["ssl"] = slice(c * 512, (c + 1) * 512)
                state["prev"] = state["cur"]
                state["cur"] = []

            def emit_o(t):
                # 4 trailing O matmuls for the previous chunk at local slot t
                pc, prev_e, o_ps = state["c"] - 1, state["prev"], state["o_ps"]
                st = t // 8
                for j in range(4):
                    tp = (t % 8) * 4 + j
                    nc.tensor.matmul(
                        o_ps[:, st, :],
                        prev_e[tp][:, st * P : (st + 1) * P],
                        v_sb[:, tp, :],
                        start=(tp == 0), stop=(tp == NT - 1),
                    )
                if pc == NB - 1:
                    if t % 8 == 7:
                        epi_st(pc, o_ps, t // 8)
                elif t == NT - 1:
                    state["epi_bulk"] = (pc, o_ps)

            def slot(t):
                c, ssl = state["c"], state["ssl"]
                s_ps = sp.tile([P, 512], F32, tag="s_ps")
                nc.tensor.matmul(
                    s_ps, kk[:, t * P : (t + 1) * P], qq[:, ssl],
                    start=True, stop=True,
                )
                if lanes[t] == "ACT":
                    e_sb = eap.tile([P, 512], BF16, tag="ea")
                    nc.scalar.activation(e_sb, s_ps, Exp)
                    e_bf = e_sb
                else:
                    e_sb = edp.tile([P, 512], I16, tag="ed")
                    nc.vector.tensor_scalar(e_sb, s_ps, EXP_L, EXP_C, Mult, Add)
                    e_bf = e_sb.bitcast(BF16)
                state["cur"].append(e_bf)
                if state["prev"] is not None:
                    if t == 0:
                        state["o_ps"] = op.tile(
                            [P, NST, HD + 1], F32, tag="o_ps", name="o_ps"
                        )
                    emit_o(t)
                state["nslot"] += 1
                if state.get("epi_bulk") and state["nslot"] % NT == bulkslot:
                    pc_, ops_ = state.pop("epi_bulk")
                    epi_bulk(pc_, ops_)
                if deferred and state["nslot"] % 8 == popslot:
                    deferred.pop(0)()

            def drain():
                # trailing O matmuls + epilogue for the final chunk
                state["prev"] = state["cur"]
                state["c"] += 1
                for t in range(NT):
                    if t == 0:
                        state["o_ps"] = op.tile(
                            [P, NST, HD + 1], F32, tag="o_ps", name="o_ps"
                        )
                        if state.get("epi_bulk"):
                            pc_, ops_ = state.pop("epi_bulk")
                            epi_bulk(pc_, ops_)
                    emit_o(t)
                    if deferred and t in ((10, 14, 18, 22, 26, 30), (12, 16, 20, 24, 28, 31), (8, 10, 12, 14, 16, 18), (14, 17, 20, 23, 26, 29))[dpops]:
                        deferred.pop(0)()

            # ---- phase B+C0: projection interleaved with chunk 0 ----
            start_chunk(0)
            for b in range(NB):
                ssl = slice(b * 512, (b + 1) * 512)
                qk_ps = sp.tile([P, 512], F32, tag="s_ps")
                for a in range(KT):
                    nc.tensor.matmul(
                        qk_ps, wqk_sb[:, a, :], xt_sb[:, a, ssl],
                        start=(a == 0), stop=(a == KT - 1),
                    )
                nc.scalar.copy(qq[:, ssl], qk_ps[:HD, :])
                nc.vector.tensor_copy(kk[:, ssl], qk_ps[HD:, :])
                v_ps = yp.tile([P, D], F32, tag="y_ps")
                for i in range(4):
                    t = b * 4 + i
                    tsl = slice(t * P, (t + 1) * P)
                    for a in range(KT):
                        nc.tensor.matmul(
                            v_ps[:, i * HD : (i + 1) * HD],
                            xt_sb[:, a, tsl], wv_sb[:, a, :],
                            start=(a == 0), stop=(a == KT - 1),
                        )
                v_ps_v = v_ps.rearrange("p (i d) -> p i d", d=HD)[:, :4, :]
                if b % 2 == 0:
                    nc.scalar.copy(v_sb[:, b * 4 : (b + 1) * 4, :HD], v_ps_v)
                else:
                    nc.vector.tensor_copy(v_sb[:, b * 4 : (b + 1) * 4, :HD], v_ps_v)
                # chunk-0 slots for the previous block's t-tiles
                if b >= 1:
                    for t in range((b - 1) * 4, b * 4):
                        slot(t)
            for t in range((NB - 1) * 4, NB * 4):
                slot(t)

            # ---- phase C: chunks 1..7 (same pipeline, no drain between) ----
            for c in range(1, NB):
                start_chunk(c)
                for t in range(NT):
                    slot(t)
            drain()
            while deferred:
                deferred.pop(0)()

    nc.compile()
    return nc


def run(inputs, trace=False, **build_kwargs):
    x = np.asarray(inputs["x"], dtype=np.float32)
    q_param = np.asarray(inputs["q_param"], dtype=np.float32)
    k_param = np.asarray(inputs["k_param"], dtype=np.float32)
    v_param = np.asarray(inputs["v_param"], dtype=np.float32)
    p_param = np.asarray(inputs["p_param"], dtype=np.float32)

    bf = ml_dtypes.bfloat16
    xt = np.ascontiguousarray(x[0].T).astype(bf)
    ident = np.tile(np.eye(P, dtype=np.float32), (1, 4)).astype(bf)
    in_maps = []
    for h in range(H):
        wqk = np.concatenate(
            [q_param[:, h, :] * SCALE, k_param[:, h, :]], axis=1
        )
        in_maps.append(
            {
                "xt": xt,
                "wqk": np.ascontiguousarray(wqk).astype(bf),
                "wv": np.ascontiguousarray(v_param[:, h, :]).astype(bf),
                "wp": np.ascontiguousarray(p_param[h]).astype(bf),
                "ident": ident,
            }
        )

    nc = build_kernel(**build_kwargs)
    res = run_bass_kernel_spmd(nc, in_maps, core_ids=list(range(H)), trace=trace)
    out = np.zeros((S, D), dtype=np.float32)
    for h in range(H):
        out += res.results[h]["y"].astype(np.float32)
    return out[None, :, :], res


def kernel(**inputs) -> np.ndarray:
    out, _ = run(inputs, trace=False)
    return out


# revision 13
# speedup vs baseline: 1.0245x; 1.0004x over previous
"""Multi-head attention (B=1, S=4096, D=512, H=8, HD=64) on 8 trn2 NeuronCores.

Sharding: one head per core. Each core projects Q/K/V for its head from a
bf16 copy of x^T, runs attention, applies its head's output projection, and
writes a full [S, D] fp32 partial; the host sums the 8 partials.

Structure (per core):
- All matmuls bf16 (fp32 PSUM accumulation). wq is pre-scaled by 1/sqrt(HD)
  on the host and packed with wk into one [D, 128] tensor so Q^T and K^T come
  out of one matmul stream ([128, 512] PSUM tiles: rows 0-63 Q^T, 64-127 K^T).
- Scores^T tiles [128 t, 512 s]: lhsT = K^T tile (stationary), rhs = Q^T.
- exp splits between ACT (exact, bf16 out) and DVE (Schraudolph: one
  tensor_scalar producing int16 bits that are the bf16 encoding of e^s).
- O accumulation is E-stationary: o_ps[s-tile, 65] += E_tile^T @ [V | 1];
  the softmax denominator Z lands on column 64, per-s on partitions.
- Epilogue per s-chunk: recip(Z), O -> bf16 (padded to 128 cols), DMA
  transpose to O^T, y_ps = O^T.T @ wp, scale by 1/Z into SBUF, DMA out.
  All deferred into the next chunk's instruction stream.
- Chunk 0 of the attention loop is interleaved with the projection phase
  (each score tile only needs K^T tiles already projected), hiding the
  x-load and projection behind chunk-0 lane work.
"""

import numpy as np
import ml_dtypes

import concourse.bacc as bacc
import concourse.mybir as mybir
import concourse.tile as tile
from concourse.bass_utils import run_bass_kernel_spmd

S = 4096
D = 512
HD = 64
H = 8
P = 128
KT = D // P            # 4 c-tiles
NB = S // 512          # 8 s-blocks / s-chunks
NT = S // P            # 32 t-tiles
NST = 512 // P         # 4 s-tiles per chunk
SCALE = HD ** -0.5

F32 = mybir.dt.float32
BF16 = mybir.dt.bfloat16
I16 = mybir.dt.int16

EXP_L = 128.0 / float(np.log(2.0))     # schraudolph multiplier for bf16 bits
EXP_C = 16256.0 - 5.5                  # bias (127<<7), centered


def build_kernel(n_dve=15, ebufs=37, n_warm=6, popslot=4, bulkslot=2, loff=0, dpops=0):
    """n_dve: DVE (approx-exp) t-tiles out of NT=32 per chunk."""
    nc = bacc.Bacc(
        "TRN2",
        target_bir_lowering=False,
        debug=False,
        enable_asserts=False,
        num_devices=H,
    )

    xt = nc.dram_tensor("xt", [D, S], BF16, kind="ExternalInput").ap()
    wqk = nc.dram_tensor("wqk", [D, P], BF16, kind="ExternalInput").ap()
    wv = nc.dram_tensor("wv", [D, HD], BF16, kind="ExternalInput").ap()
    wp = nc.dram_tensor("wp", [HD, D], BF16, kind="ExternalInput").ap()
    ident = nc.dram_tensor("ident", [P, 4 * P], BF16, kind="ExternalInput").ap()
    y = nc.dram_tensor("y", [S, D], BF16, kind="ExternalOutput").ap()

    Exp = mybir.ActivationFunctionType.Exp
    Mult = mybir.AluOpType.mult
    Add = mybir.AluOpType.add

    # lane pattern per chunk: n_dve DVE t-tiles spread among ACT t-tiles
    lanes = ["ACT"] * NT
    if n_dve > 0:
        step = NT / n_dve
        for i in range(n_dve):
            lanes[min(NT - 1, int(i * step) + loff)] = "DVE"

    with tile.TileContext(nc) as tc:
        with (
            tc.tile_pool(name="const", bufs=1) as cp,
            tc.tile_pool(name="ea", bufs=ebufs) as eap,
            tc.tile_pool(name="ed", bufs=ebufs) as edp,
            tc.tile_pool(name="ob", bufs=5) as obp,
            tc.tile_pool(name="otb", bufs=5) as otbp,
            tc.tile_pool(name="ys", bufs=3) as ysp,
            tc.tile_pool(name="xtp", bufs=1) as xtp,
            tc.tile_pool(name="sp", bufs=4, space="PSUM") as sp,
            tc.tile_pool(name="op", bufs=2, space="PSUM") as op,
            tc.tile_pool(name="yp", bufs=1, space="PSUM") as yp,
            tc.tile_pool(name="trp", bufs=1, space="PSUM") as trp,
        ):
            wqk_sb = cp.tile([P, KT, P], BF16, tag="wqk")
            id_sb = cp.tile([P, 4 * P], BF16, tag="ident")
            wv_sb = cp.tile([P, KT, HD], BF16, tag="wv")
            wp_sb = cp.tile([HD, D], BF16, tag="wp")
            qq = cp.tile([HD, S], BF16, tag="qq")      # Q^T (pre-scaled)
            kk = cp.tile([HD, S], BF16, tag="kk")      # K^T
            v_sb = cp.tile([P, NT, HD + 1], BF16, tag="v")
            rz_sb = cp.tile([P, NT, 1], F32, tag="rz")  # 1/Z per s-tile
            xt_sb = xtp.tile([P, KT, S], BF16, tag="xt")

            wsrc = cp.tile([P, 512], BF16, tag="wsrc")
            nc.gpsimd.memset(wsrc, 1.0)
            xt_r = xt.rearrange("(a p) s -> p a s", p=P)
            nc.sync.dma_start(xt_sb[:, :, 0:512], xt_r[:, :, 0:512])
            nc.sync.dma_start(wqk_sb, wqk.rearrange("(a p) d -> p a d", p=P))
            nc.sync.dma_start(id_sb, ident)
            nc.sync.dma_start(wv_sb, wv.rearrange("(a p) d -> p a d", p=P))
            for b in (1, 2, 3):
                ssl = slice(b * 512, (b + 1) * 512)
                nc.sync.dma_start(xt_sb[:, :, ssl], xt_r[:, :, ssl])
            for b in (4, 6):
                ssl = slice(b * 512, (b + 2) * 512)
                nc.sync.dma_start(xt_sb[:, :, ssl], xt_r[:, :, ssl])
            nc.sync.dma_start(wp_sb, wp)
            nc.gpsimd.memset(v_sb[:, :, HD : HD + 1], 1.0)

            # PE warm-up: matmuls on a memset tile need no DMA, so the PE
            # clock ramp starts immediately while the first x block lands.
            for _ in range(n_warm):
                warm_ps = sp.tile([P, 512], F32, tag="s_ps", name="warm_ps")
                nc.tensor.matmul(
                    warm_ps, wsrc[:, :P], wsrc, start=True, stop=True
                )

            deferred = []
            epi = {}

            def epi_bulk(c, o_ps):
                o_sb = obp.tile([P, NST, P], BF16, tag="ob", name="o_sb")
                ot_sb = otbp.tile([P, NST, P], BF16, tag="otb", name="ot_sb")
                nc.vector.reciprocal(
                    rz_sb[:, c * NST : (c + 1) * NST, 0], o_ps[:, :, HD]
                )
                nc.vector.tensor_copy(o_sb[:, :, :HD], o_ps[:, :, :HD])
                for st in range(NST):
                    nc.sync.dma_start(
                        ot_sb[:, st, :], o_sb[:, st, :], transpose=True
                    )

                def make_part2(sp0):
                    def part2():
                        ys = ysp.tile([P, 2, D], BF16, tag="ys")
                        for i in range(2):
                            st = sp0 + i
                            y_ps = yp.tile([P, D], F32, tag="y_ps")
                            nc.tensor.matmul(
                                y_ps, ot_sb[:HD, st, :], wp_sb,
                                start=True, stop=True,
                            )
                            rz = rz_sb[:, c * NST + st, :]
                            if st % 2 == 0:
                                nc.vector.tensor_scalar_mul(ys[:, i, :], y_ps, rz)
                            else:
                                nc.scalar.mul(ys[:, i, :], y_ps, rz)
                        yd = y[c * 512 + sp0 * P : c * 512 + (sp0 + 2) * P, :]
                        nc.sync.dma_start(yd.rearrange("(a p) d -> p a d", p=P), ys)
                    return part2

                for sp0 in (0, 2):
                    deferred.append(make_part2(sp0))

            def epi_st(c, o_ps, st):
                # per-s-tile epilogue, fired as soon as O group st closes
                last = c == NB - 1
                if st == 0:
                    epi["o_sb"] = obp.tile([P, NST, P], BF16, tag="ob", name="o_sb")
                    epi["ot_sb"] = otbp.tile(
                        [P, NST, P], BF16, tag="otb", name="ot_sb"
                    )
                    if last:
                        epi["tr"] = trp.tile(
                            [HD, NST, P], BF16, tag="tr", name="tr"
                        )
                o_sb, ot_sb = epi["o_sb"], epi["ot_sb"]
                nc.vector.reciprocal(
                    rz_sb[:, c * NST + st, 0:1], o_ps[:, st, HD : HD + 1]
                )
                if st % 2 == 0:
                    nc.vector.tensor_copy(o_sb[:, st, :HD], o_ps[:, st, :HD])
                else:
                    nc.scalar.copy(o_sb[:, st, :HD], o_ps[:, st, :HD])
                if last:
                    tr_ps = epi["tr"]
                    nc.tensor.transpose(
                        tr_ps[:, st, :], o_sb[:, st, :HD], id_sb[:, :P]
                    )
                    if st % 2 == 0:
                        nc.vector.tensor_copy(ot_sb[:HD, st, :], tr_ps[:, st, :])
                    else:
                        nc.scalar.copy(ot_sb[:HD, st, :], tr_ps[:, st, :])
                else:
                    nc.sync.dma_start(
                        ot_sb[:, st, :], o_sb[:, st, :], transpose=True
                    )

                def part2():
                    sp0 = st - 1
                    ys = ysp.tile([P, 2, D], BF16, tag="ys")
                    split = last and st == NST - 1
                    for i in range(2):
                        sti = sp0 + i
                        if last:
                            y_ps = sp.tile([P, D], F32, tag="s_ps", name="y_ps_t")
                        else:
                            y_ps = yp.tile([P, D], F32, tag="y_ps")
                        nc.tensor.matmul(
                            y_ps, ot_sb[:HD, sti, :], wp_sb,
                            start=True, stop=True,
                        )
                        rz = rz_sb[:, c * NST + sti, :]
                        if sti % 2 == 0:
                            nc.vector.tensor_scalar_mul(ys[:, i, :], y_ps, rz)
                        else:
                            nc.scalar.mul(ys[:, i, :], y_ps, rz)
                        if split:
                            # issue each final write as soon as its scale is
                            # done: the very last transfer is half-size
                            nc.sync.dma_start(
                                y[c * 512 + sti * P : c * 512 + (sti + 1) * P, :],
                                ys[:, i, :],
                            )
                    if not split:
                        yd = y[c * 512 + sp0 * P : c * 512 + (sp0 + 2) * P, :]
                        nc.sync.dma_start(yd.rearrange("(a p) d -> p a d", p=P), ys)

                if st % 2 == 1:
                    deferred.append(part2)

            # slot machinery: O matmuls trail the scores by one full chunk
            # so the four o_ps accumulation groups run SEQUENTIALLY (psum
            # banks cannot hold two concurrently-open accumulation groups).
            state = {"nslot": 0, "prev": None, "cur": None, "o_ps": None}

            def start_chunk(c):
                state["c"] = c
                state["ssl"] = slice(c * 512, (c + 1) * 512)
                state["prev"] = state["cur"]
                state["cur"] = []

            def emit_o(t):
                # 4 trailing O matmuls for the previous chunk at local slot t
                pc, prev_e, o_ps = state["c"] - 1, state["prev"], state["o_ps"]
                st = t // 8
                for j in range(4):
                    tp = (t % 8) * 4 + j
                    nc.tensor.matmul(
                        o_ps[:, st, :],
                        prev_e[tp][:, st * P : (st + 1) * P],
                        v_sb[:, tp, :],
                        start=(tp == 0), stop=(tp == NT - 1),
                    )
                if pc == NB - 1:
                    if t % 8 == 7:
                        epi_st(pc, o_ps, t // 8)
                elif t == NT - 1:
                    state["epi_bulk"] = (pc, o_ps)

            def slot(t):
                c, ssl = state["c"], state["ssl"]
                s_ps = sp.tile([P, 512], F32, tag="s_ps")
                nc.tensor.matmul(
                    s_ps, kk[:, t * P : (t + 1) * P], qq[:, ssl],
                    start=True, stop=True,
                )
                if lanes[t] == "ACT":
                    e_sb = eap.tile([P, 512], BF16, tag="ea")
                    nc.scalar.activation(e_sb, s_ps, Exp)
                    e_bf = e_sb
                else:
                    e_sb = edp.tile([P, 512], I16, tag="ed")
                    nc.vector.tensor_scalar(e_sb, s_ps, EXP_L, EXP_C, Mult, Add)
                    e_bf = e_sb.bitcast(BF16)
                state["cur"].append(e_bf)
                if state["prev"] is not None:
                    if t == 0:
                        state["o_ps"] = op.tile(
                            [P, NST, HD + 1], F32, tag="o_ps", name="o_ps"
                        )
                    emit_o(t)
                state["nslot"] += 1
                if state.get("epi_bulk") and state["nslot"] % NT == bulkslot:
                    pc_, ops_ = state.pop("epi_bulk")
                    epi_bulk(pc_, ops_)
                if deferred and state["nslot"] % 8 == popslot:
                    deferred.pop(0)()

            def drain():
                # trailing O matmuls + epilogue for the final chunk
                state["prev"] = state["cur"]
                state["c"] += 1
                for t in range(NT):
                    if t == 0:
                        state["o_ps"] = op.tile(
                            [P, NST, HD + 1], F32, tag="o_ps", name="o_ps"
                        )
                        if state.get("epi_bulk"):
                            pc_, ops_ = state.pop("epi_bulk")
                            epi_bulk(pc_, ops_)
                    emit_o(t)
                    if deferred and t in ((10, 14, 18, 22, 26, 30), (12, 16, 20, 24, 28, 31), (8, 10, 12, 14, 16, 18), (14, 17, 20, 23, 26, 29))[dpops]:
                        deferred.pop(0)()

            # ---- phase B+C0: projection interleaved with chunk 0 ----
            start_chunk(0)
            for b in range(NB):
                ssl = slice(b * 512, (b + 1) * 512)
                qk_ps = sp.tile([P, 512], F32, tag="s_ps")
                for a in range(KT):
                    nc.tensor.matmul(
                        qk_ps, wqk_sb[:, a, :], xt_sb[:, a, ssl],
                        start=(a == 0), stop=(a == KT - 1),
                    )
                nc.scalar.copy(qq[:, ssl], qk_ps[:HD, :])
                nc.vector.tensor_copy(kk[:, ssl], qk_ps[HD:, :])
                v_ps = yp.tile([P, D], F32, tag="y_ps")
                for i in range(4):
                    t = b * 4 + i
                    tsl = slice(t * P, (t + 1) * P)
                    for a in range(KT):
                        nc.tensor.matmul(
                            v_ps[:, i * HD : (i + 1) * HD],
                            xt_sb[:, a, tsl], wv_sb[:, a, :],
                            start=(a == 0), stop=(a == KT - 1),
                        )
                v_ps_v = v_ps.rearrange("p (i d) -> p i d", d=HD)[:, :4, :]
                if b % 2 == 0:
                    nc.scalar.copy(v_sb[:, b * 4 : (b + 1) * 4, :HD], v_ps_v)
                else:
                    nc.vector.tensor_copy(v_sb[:, b * 4 : (b + 1) * 4, :HD], v_ps_v)
                # chunk-0 slots for the previous block's t-tiles
                if b >= 1:
                    for t in range((b - 1) * 4, b * 4):
                        slot(t)
            for t in range((NB - 1) * 4, NB * 4):
                slot(t)

            # ---- phase C: chunks 1..7 (same pipeline, no drain between) ----
            for c in range(1, NB):
                start_chunk(c)
                for t in range(NT):
                    slot(t)
            drain()
            while deferred:
                deferred.pop(0)()

    nc.compile()
    return nc


def run(inputs, trace=False, **build_kwargs):
    x = np.asarray(inputs["x"], dtype=np.float32)
    q_param = np.asarray(inputs["q_param"], dtype=np.float32)
    k_param = np.asarray(inputs["k_param"], dtype=np.float32)
    v_param = np.asarray(inputs["v_param"], dtype=np.float32)
    p_param = np.asarray(inputs["p_param"], dtype=np.float32)

    bf = ml_dtypes.bfloat16
    xt = np.ascontiguousarray(x[0].T).astype(bf)
    ident = np.tile(np.eye(P, dtype=np.float32), (1, 4)).astype(bf)
    in_maps = []
    for h in range(H):
        wqk = np.concatenate(
            [q_param[:, h, :] * SCALE, k_param[:, h, :]], axis=1
        )
        in_maps.append(
            {
                "xt": xt,
                "wqk": np.ascontiguousarray(wqk).astype(bf),
                "wv": np.ascontiguousarray(v_param[:, h, :]).astype(bf),
                "wp": np.ascontiguousarray(p_param[h]).astype(bf),
                "ident": ident,
            }
        )

    nc = build_kernel(**build_kwargs)
    res = run_bass_kernel_spmd(nc, in_maps, core_ids=list(range(H)), trace=trace)
    out = np.zeros((S, D), dtype=np.float32)
    for h in range(H):
        out += res.results[h]["y"].astype(np.float32)
    return out[None, :, :], res


def kernel(**inputs) -> np.ndarray:
    out, _ = run(inputs, trace=False)
    return out
